# revision 1
# baseline (speedup 1.0000x reference)
"""Trainium2 Bass kernel for nn_CELoss_4896262717859.

Computes, for each query column c = idx_node[k] of a sparse adjacency matrix
(diagonal zeroed), a cross-entropy-style loss over the "lower" (r < c) and
"upper" (r > c) neighbor sets:

    contrib_side(c) = [cnt>0 and poscnt==1] * (log(sum_r m exp(out_r)) - poslogit) / cnt

All per-column quantities are sums of the form sum_r adj[r,c] * w[r] for
w in {1, pos, pos*out, exp(out)} -> computed as tensor-engine matvecs with a
triangular split, per-column for ALL N columns, then gathered at idx_node on
the host (O(N+K) combine).

Sharding: columns split into 8 slabs of 1024 (one per core). Each core reads
its [8192 x 1024] int32 slab contiguously (memory roofline), casts to bf16,
and accumulates psum[12, 1024] stats = {L,U} x {ones, pos, pl_hi, pl_lo,
e_hi, e_lo}. The core's row order is rotated by 1024*core so the diagonal
block always falls in local row-tiles 0..7 -> one NEFF serves all cores; the
L/U routing of full tiles is data-driven via zero-padded weight variants.
"""

import numpy as np
import ml_dtypes

N = 8192
K = 4096
NCORES = 8
SLAB = N // NCORES        # 1024 columns per core
P = 128                   # partition / tile edge
NT = N // P               # 64 row tiles
TPC = SLAB // P           # 8 diagonal tiles per core
NW = 6                    # weights per side
M = 2 * NW                # 12 psum partitions (L half = 0:6, U half = 6:12)
MMN = 512                 # max matmul free size

BF16 = ml_dtypes.bfloat16

_BASS_CACHE = {}


def _build_bass():
    import concourse.tile as tile
    import concourse.mybir as mybir
    from concourse import bacc

    # Bacc (not raw Bass): its compile() runs generate_event_semaphores,
    # which splits multi-sem waits — TRN2 instructions hold at most one.
    nc = bacc.Bacc("TRN2")
    adj = nc.dram_tensor("adj", [N, SLAB], mybir.dt.int32, kind="ExternalInput")
    wmat = nc.dram_tensor(
        "wmat", [P, (NT + TPC) * M], mybir.dt.bfloat16, kind="ExternalInput"
    )
    masks = nc.dram_tensor("masks", [P, 2 * P], mybir.dt.bfloat16, kind="ExternalInput")
    stats = nc.dram_tensor("stats", [M, SLAB], mybir.dt.float32, kind="ExternalOutput")

    with tile.TileContext(nc) as tc:
        with (
            tc.tile_pool(name="singles", bufs=1) as singles,
            # bufs multiple of 8 matches the 8-queue HWDGE round-robin: the
            # slot-reuse predecessor of each adj DMA lands on the SAME queue,
            # so its WAW ordering is implicit and the DMA carries a single
            # sync-wait (the DMA ISA struct has room for only one).
            tc.tile_pool(name="io", bufs=8) as io_pool,
            tc.tile_pool(name="bf", bufs=6) as bf_pool,
            tc.tile_pool(name="diag", bufs=TPC) as diag_pool,
            tc.tile_pool(name="psum", bufs=1, space="PSUM") as psum_pool,
        ):
            # issue the first two adjacency DMAs before anything else so the
            # HBM-saturated stream (the critical path) starts ~1.3us earlier;
            # the small wmat/masks loads slot in behind them.
            pre = {}
            for j in range(2):
                t = io_pool.tile([P, SLAB], mybir.dt.int32, tag="adj_i")
                nc.sync.dma_start(out=t, in_=adj[j * P : (j + 1) * P, :])
                pre[j] = t

            wsb = singles.tile([P, (NT + TPC) * M], mybir.dt.bfloat16)
            nc.sync.dma_start(out=wsb, in_=wmat[:, :])
            msb_raw = singles.tile([P, 2 * P], mybir.dt.bfloat16)
            nc.sync.dma_start(out=msb_raw, in_=masks[:, :])
            # Re-produce the masks on DVE: the DVE TensorTensor ISA struct has
            # room for a single sync-wait, so the diag-mask multiplies must
            # only ever depend on DVE-produced operands (one self-sem wait).
            msb = singles.tile([P, 2 * P], mybir.dt.bfloat16)
            nc.vector.tensor_copy(msb, msb_raw)

            # one psum tile per 512-col bank: Tile's RAW deps are whole-tile,
            # so separate tiles let bank A's copy-out overlap bank B's final
            # matmuls
            accs = [
                psum_pool.tile(
                    [M, MMN], mybir.dt.float32, tag=f"acc{b}", name=f"acc{b}"
                )
                for b in range(SLAB // MMN)
            ]

            def wv(v):
                return wsb[:, v * M : (v + 1) * M]

            # start=True zeroes the ENTIRE psum bank(s) a matmul touches, so
            # (a) every matmul stays inside one 512-col bank, (b) exactly the
            # first matmul touching each bank carries start=True.
            bank_started = [False] * (SLAB // MMN)

            def mm_seg(w, rhs_slice, a, b, stop=False):
                bank = a // MMN
                assert b <= (bank + 1) * MMN
                nc.tensor.matmul(
                    accs[bank][:, a - bank * MMN : b - bank * MMN], w, rhs_slice,
                    start=not bank_started[bank], stop=stop,
                    skip_group_check=True,
                )
                bank_started[bank] = True

            def mm(w, rhs_full, a, b, stop=False):
                while a < b:
                    e = min(b, (a // MMN + 1) * MMN)
                    mm_seg(w, rhs_full[:, a:e], a, e, stop=stop)
                    a = e

            for j in range(NT):
                last = j == NT - 1
                if j in pre:
                    adj_i = pre.pop(j)
                else:
                    adj_i = io_pool.tile([P, SLAB], mybir.dt.int32, tag="adj_i")
                    if last:
                        # split the final load so its first half (and the
                        # bank-A matmul) overlaps the second half's transfer
                        nc.sync.dma_start(
                            out=adj_i[:, 0:MMN], in_=adj[j * P :, 0:MMN]
                        )
                        nc.sync.dma_start(
                            out=adj_i[:, MMN:], in_=adj[j * P :, MMN:]
                        )
                    else:
                        nc.sync.dma_start(out=adj_i, in_=adj[j * P : (j + 1) * P, :])
                adj_b = bf_pool.tile([P, SLAB], mybir.dt.bfloat16)
                if last:
                    # fine-grained pipeline on the final tile: shortest
                    # latency from last-byte-arrival to last matmul, with
                    # the final chunk halved again to 128 cols
                    bounds = [0, 256, 512, 768, 896, SLAB]
                    for s, e in zip(bounds[:-1], bounds[1:]):
                        nc.vector.tensor_copy(adj_b[:, s:e], adj_i[:, s:e])
                        mm(wv(j), adj_b, s, e, stop=(e == SLAB))
                    continue
                nc.vector.tensor_copy(adj_b, adj_i)

                if j < TPC:
                    WL, WU = wv(j), wv(NT + j)
                    c0, c1 = j * P, (j + 1) * P
                    mlo = diag_pool.tile([P, P], mybir.dt.bfloat16)
                    nc.vector.tensor_mul(mlo, adj_b[:, c0:c1], msb[:, 0:P])
                    mup = diag_pool.tile([P, P], mybir.dt.bfloat16)
                    nc.vector.tensor_mul(mup, adj_b[:, c0:c1], msb[:, P : 2 * P])
                    # full columns left of the diag block: rows > cols -> U
                    mm(WU, adj_b, 0, c0)
                    mm_seg(WL, mlo, c0, c1)
                    mm_seg(WU, mup, c0, c1)
                    # full columns right of the diag block: rows < cols -> L
                    mm(WL, adj_b, c1, SLAB)
                else:
                    mm(wv(j), adj_b, 0, SLAB, stop=last)

            # per-bank copy-out: bank A's copy/DMA overlap the final bank-B
            # matmul (ACT reads psum bank A while PE writes bank B); bank B's
            # copy is split across ACT and DVE so the two halves run in
            # parallel on the critical tail
            out_sb = singles.tile([M, SLAB], mybir.dt.float32)
            nc.scalar.copy(out_sb[:, 0:MMN], accs[0])
            nc.sync.dma_start(out=stats[:, 0:MMN], in_=out_sb[:, 0:MMN])
            half = MMN // 2
            nc.scalar.copy(out_sb[:, MMN : MMN + half], accs[1][:, 0:half])
            nc.vector.tensor_copy(out_sb[:, MMN + half :], accs[1][:, half:])
            nc.sync.dma_start(out=stats[:, MMN:], in_=out_sb[:, MMN:])

    nc.compile()
    return nc


def _split_bf16(v):
    hi = v.astype(BF16)
    lo = (v - hi.astype(np.float64)).astype(BF16)
    return hi, lo


def _host_prep(outputs, targets):
    """Per-row weight table Wside [N, 6] bf16 and per-core inputs."""
    out = np.asarray(outputs, np.float64).reshape(-1)
    pos = (np.asarray(targets).reshape(-1) != 0).astype(np.float64)
    pl_hi, pl_lo = _split_bf16(pos * out)
    e_hi, e_lo = _split_bf16(np.exp(out))
    wside = np.stack(
        [
            np.ones(N, BF16),
            pos.astype(BF16),
            pl_hi,
            pl_lo,
            e_hi,
            e_lo,
        ],
        axis=1,
    ).astype(BF16)  # [N, 6]

    # triangular masks for the diagonal 128-block (strict)
    ri = np.arange(P)[:, None]
    ci = np.arange(P)[None, :]
    masks = np.concatenate(
        [(ri < ci).astype(BF16), (ri > ci).astype(BF16)], axis=1
    )  # [128, 256]
    return wside, np.ascontiguousarray(masks)


def _build_wmat(wside, core):
    """Per-core weight variants [128, (64+8)*12] bf16.

    Variant j (j<64): weights for local row tile j (absolute tile (8*core+j)%64).
      j < 8  -> L-only variant (diag tiles; U-only twin stored at 64+j)
      j >= 8 -> single variant, L or U half per the tile's position vs the slab
    """
    w = np.zeros((P, NT + TPC, M), dtype=BF16)
    for j in range(NT):
        t = (TPC * core + j) % NT
        rows = wside[t * P : (t + 1) * P, :]  # [128, 6]
        if j < TPC:
            w[:, j, 0:NW] = rows
            w[:, NT + j, NW:M] = rows
        elif j < NT - TPC * core:
            w[:, j, NW:M] = rows  # rows above slab columns -> U
        else:
            w[:, j, 0:NW] = rows  # wrapped rows below slab columns -> L
    return np.ascontiguousarray(w.reshape(P, (NT + TPC) * M))


def _build_shard(node_adj, core):
    """Rotated column slab [N, SLAB] int32: local row rho = (abs_row - SLAB*core) mod N."""
    c0 = SLAB * core
    cols = node_adj[:, c0 : c0 + SLAB]
    if core == 0:
        return np.ascontiguousarray(cols, dtype=np.int32)
    return np.ascontiguousarray(
        np.concatenate([cols[c0:], cols[:c0]], axis=0), dtype=np.int32
    )


def _combine(stats_list, idx_node):
    """stats_list: per-core [12, SLAB] f32 -> scalar loss (f64 math)."""
    full = np.concatenate([np.asarray(s, np.float64) for s in stats_list], axis=1)

    def side_contrib(x):
        cnt, poscnt = x[0], x[1]
        poslogit = x[2] + x[3]
        sumexp = x[4] + x[5]
        valid = (cnt > 0.5) & (np.abs(poscnt - 1.0) < 0.25)
        lse = np.log(np.where(valid, np.maximum(sumexp, 1e-300), 1.0))
        return np.where(valid, (lse - poslogit) / np.maximum(cnt, 1.0), 0.0)

    contrib = side_contrib(full[0:NW]) + side_contrib(full[NW:M])
    idx = np.asarray(idx_node).reshape(-1).astype(np.int64)
    return np.array(contrib[idx].sum(), dtype=np.float32)


def _ensure_axon_hooks_stub():
    """bass_utils imports antenv.axon_hooks when tracing is requested via
    env; the module is absent on some images. Provide a no-op stub so the
    import never crashes (hook=None -> bass_utils skips tracing)."""
    import sys
    import types

    try:
        import antenv.axon_hooks  # noqa: F401
    except ImportError:
        mod = types.ModuleType("antenv.axon_hooks")
        state = {"hook": None}
        mod.set_axon_ntff_profile_hook = lambda h: state.__setitem__("hook", h)
        mod.get_axon_ntff_profile_hook = lambda: state["hook"]
        sys.modules["antenv.axon_hooks"] = mod


def _device_stats(in_maps):
    _ensure_axon_hooks_stub()
    from concourse.bass_utils import run_bass_kernel_spmd

    if "nc" not in _BASS_CACHE:
        _BASS_CACHE["nc"] = _build_bass()
    last_exc = None
    for attempt in range(4):
        try:
            res = run_bass_kernel_spmd(
                _BASS_CACHE["nc"], in_maps, core_ids=list(range(NCORES))
            )
            return [r["stats"] for r in res.results]
        except Exception as e:  # transient NRT/accelerator hiccups
            last_exc = e
            try:
                # a fresh PJRT client usually recovers a transiently
                # "unrecoverable" accelerator; mirrors a process restart
                import jax
                import jax.extend.backend as _jeb

                jax.clear_caches()
                _jeb.clear_backends()
            except Exception:
                pass
            import time

            time.sleep(2.0 * (attempt + 1))
    raise last_exc


def _sim_stats(in_maps):
    """Numpy emulation of the device kernel (same inputs), for logic validation."""
    outs = []
    for m in in_maps:
        adj = m["adj"].astype(np.float32)
        w = m["wmat"].reshape(P, NT + TPC, M).astype(np.float32)
        msk = m["masks"].astype(np.float32)
        lowm, upm = msk[:, 0:P], msk[:, P:]
        acc = np.zeros((M, SLAB), np.float32)
        for j in range(NT):
            tile = adj[j * P : (j + 1) * P, :]
            if j < TPC:
                WL, WU = w[:, j, :], w[:, NT + j, :]
                c0, c1 = j * P, (j + 1) * P
                acc[:, :c0] += WU.T @ tile[:, :c0]
                acc[:, c0:c1] += WL.T @ (tile[:, c0:c1] * lowm)
                acc[:, c0:c1] += WU.T @ (tile[:, c0:c1] * upm)
                acc[:, c1:] += WL.T @ tile[:, c1:]
            else:
                acc += w[:, j, :].T @ tile
        outs.append(acc)
    return outs


def kernel(outputs, targets, node_adj, idx_node, _simulate=False):
    node_adj = np.asarray(node_adj)
    wside, masks = _host_prep(outputs, targets)
    in_maps = [
        {
            "adj": _build_shard(node_adj, d),
            "wmat": _build_wmat(wside, d),
            "masks": masks,
        }
        for d in range(NCORES)
    ]
    stats = _sim_stats(in_maps) if _simulate else _device_stats(in_maps)
    return _combine(stats, idx_node)



# revision 2
# speedup vs baseline: 3.2844x; 3.2844x over previous
"""Trainium2 Bass kernel for nn_CELoss_4896262717859 (v2: gathered fp8 columns).

For each query column c = idx_node[k] of a sparse adjacency matrix (diagonal
zeroed), a cross-entropy-style loss over the "lower" (r < c) and "upper"
(r > c) neighbor sets:

    contrib_side(c) = [cnt>0 and poscnt==1] * (log(sum_r m exp(out_r)) - poslogit) / cnt

All per-column quantities are sums  sum_r adj[r,c] * w[r]  for
w in {1, pos, pos*out, exp(out)} -> tensor-engine matvecs with a triangular
(L/U) split, computed ONLY for the distinct idx_node columns (~3218 of 8192),
then combined with multiplicities on the host (O(N+K)).

Sharding: core d handles the distinct query columns falling in column slab
[1024d, 1024(d+1)).  Within a slab, columns are bucketed by the 128-row block
containing their diagonal (the "mixed" block); each of the 8 buckets is padded
to a fixed 64 slots -> exactly 512 column slots per core, so ONE compiled
program (fixed matmul ranges) serves every core and any input.  Rows are
rotated by 1024d so the mixed blocks always land in local row-tiles 0..7.

The adjacency is shipped as fp8e4 (0/1 exact): 4.2 MB/core instead of the
baseline's 32 MB int32 -> memory roofline ~12us.  The mixed 128-row block of
each column is pre-masked on the host: its lower part (rows < c) replaces the
block in the main slab (covered by the L matmul), its upper part goes to a
small separate diagu[128,512] operand (covered by one extra 64-wide matmul
per diagonal tile).  No on-device casts or mask multiplies remain: weights
(bf16, hi/lo split) stream against fp8 adjacency directly.

Any bucket overflow beyond 64 distinct columns (never happens for uniform
idx_node; p<1% per bucket) falls back to a tiny host-side computation for the
overflowed columns only.
"""

import numpy as np
import ml_dtypes

N = 8192
K = 4096
NCORES = 8
SLAB = N // NCORES        # 1024 columns per slab
P = 128                   # partition / tile edge
NT = N // P               # 64 row tiles
TPC = SLAB // P           # 8 diagonal (mixed) tiles per core
NW = 6                    # weights per side
M = 2 * NW                # 12 psum partitions (L half = 0:6, U half = 6:12)
CAP = 512                 # column slots per core (one psum bank)
BCAP = CAP // TPC         # 64 slots per 128-row bucket
CHT = 8                   # row tiles per DMA chunk
NCH = NT // CHT           # 8 chunks

BF16 = ml_dtypes.bfloat16
FP8 = ml_dtypes.float8_e4m3fn

_BASS_CACHE = {}


def _build_bass():
    import concourse.tile as tile
    import concourse.mybir as mybir
    from concourse import bacc

    # Bacc (not raw Bass): its compile() runs generate_event_semaphores,
    # which splits multi-sem waits — TRN2 instructions hold at most one.
    nc = bacc.Bacc("TRN2")
    adj = nc.dram_tensor("adj", [P, NT * CAP], mybir.dt.float8e4, kind="ExternalInput")
    diagu = nc.dram_tensor("diagu", [P, CAP], mybir.dt.float8e4, kind="ExternalInput")
    wmat = nc.dram_tensor(
        "wmat", [P, (NT + TPC) * M], mybir.dt.bfloat16, kind="ExternalInput"
    )
    stats = nc.dram_tensor("stats", [M, CAP], mybir.dt.float32, kind="ExternalOutput")

    with tile.TileContext(nc) as tc:
        with (
            tc.tile_pool(name="singles", bufs=1) as singles,
            tc.tile_pool(name="psum", bufs=1, space="PSUM") as psum_pool,
        ):
            # adjacency chunk 0 first: it gates the first matmuls; the small
            # wmat/diagu loads slot in behind it.
            chunks = []
            ch0 = singles.tile([P, CHT * CAP], mybir.dt.float8e4, name="ch0")
            nc.sync.dma_start(out=ch0, in_=adj[:, 0 : CHT * CAP])
            chunks.append(ch0)
            wsb = singles.tile([P, (NT + TPC) * M], mybir.dt.bfloat16)
            nc.sync.dma_start(out=wsb, in_=wmat[:, :])
            dsb = singles.tile([P, CAP], mybir.dt.float8e4)
            nc.sync.dma_start(out=dsb, in_=diagu[:, :])
            for ci in range(1, NCH):
                t = singles.tile([P, CHT * CAP], mybir.dt.float8e4, name=f"ch{ci}")
                nc.sync.dma_start(out=t, in_=adj[:, ci * CHT * CAP : (ci + 1) * CHT * CAP])
                chunks.append(t)

            acc = psum_pool.tile([M, CAP], mybir.dt.float32, name="acc")

            def wv(v):
                return wsb[:, v * M : (v + 1) * M]

            def mm(w, rhs, a, b, start=False, stop=False):
                nc.tensor.matmul(
                    acc[:, a:b], w, rhs, start=start, stop=stop,
                    skip_group_check=True,
                )

            for j in range(NT):
                rhs = chunks[j // CHT][:, (j % CHT) * CAP : (j % CHT + 1) * CAP]
                last = j == NT - 1
                if j < TPC:
                    c0 = BCAP * j
                    # rows < c for slots right of (and inside, pre-masked) the
                    # mixed bucket -> L weights
                    mm(wv(j), rhs[:, c0:CAP], c0, CAP, start=(j == 0))
                    if j > 0:
                        # rows > c for slots left of the mixed bucket -> U
                        mm(wv(NT + j), rhs[:, 0:c0], 0, c0)
                    # upper part of the mixed bucket itself
                    mm(wv(NT + j), dsb[:, c0 : c0 + BCAP], c0, c0 + BCAP)
                else:
                    mm(wv(j), rhs, 0, CAP, stop=last)

            out_sb = singles.tile([M, CAP], mybir.dt.float32)
            half = CAP // 2
            nc.scalar.copy(out_sb[:, 0:half], acc[:, 0:half])
            nc.vector.tensor_copy(out_sb[:, half:], acc[:, half:])
            nc.sync.dma_start(out=stats[:, :], in_=out_sb)

    nc.compile()
    return nc


def _split_bf16(v):
    hi = v.astype(BF16)
    lo = (v - hi.astype(np.float64)).astype(BF16)
    return hi, lo


def _make_wside(outputs, targets):
    """Per-row weight table [N, 6] bf16: {1, pos, pos*out hi/lo, exp(out) hi/lo}."""
    out = np.asarray(outputs, np.float64).reshape(-1)
    pos = (np.asarray(targets).reshape(-1) != 0).astype(np.float64)
    pl_hi, pl_lo = _split_bf16(pos * out)
    e_hi, e_lo = _split_bf16(np.exp(out))
    return np.stack(
        [np.ones(N, BF16), pos.astype(BF16), pl_hi, pl_lo, e_hi, e_lo], axis=1
    ).astype(BF16)


def _build_wmat(wside, core):
    """Per-core weight variants [128, (64+8)*12] bf16.

    Variant j (j<64): weights for local row tile j (absolute tile (8*core+j)%64).
      j < 8  -> L-only variant (diag tiles; U-only twin stored at 64+j)
      j >= 8 -> single variant, L or U half per the tile's position vs the slab
    """
    w = np.zeros((P, NT + TPC, M), dtype=BF16)
    for j in range(NT):
        t = (TPC * core + j) % NT
        rows = wside[t * P : (t + 1) * P, :]
        if j < TPC:
            w[:, j, 0:NW] = rows
            w[:, NT + j, NW:M] = rows
        elif j < NT - TPC * core:
            w[:, j, NW:M] = rows  # rows above slab columns -> U
        else:
            w[:, j, 0:NW] = rows  # wrapped rows below slab columns -> L
    return np.ascontiguousarray(w.reshape(P, (NT + TPC) * M))


def _prepare(outputs, targets, node_adj, idx_node):
    """Build per-core in_maps + combine context (slot->column map, multiplicities,
    host-computed contribution of any bucket-overflow columns)."""
    node_adj = np.asarray(node_adj)
    idx = np.asarray(idx_node).reshape(-1).astype(np.int64)
    ucols, mult = np.unique(idx, return_counts=True)
    wside = _make_wside(outputs, targets)

    in_maps = []
    slot_cols = np.full((NCORES, CAP), -1, np.int64)
    overflow = []
    rows128 = np.arange(P)
    s_idx = np.arange(CAP)
    base = P * (s_idx // BCAP)  # first local row of each slot's mixed block

    for d in range(NCORES):
        lo = SLAB * d
        uc = ucols[(ucols >= lo) & (ucols < lo + SLAB)]
        cols_s = np.full(CAP, -1, np.int64)
        for b in range(TPC):
            blk = uc[(uc - lo) // P == b]
            if len(blk) > BCAP:
                overflow.extend(blk[BCAP:].tolist())
                blk = blk[:BCAP]
            cols_s[BCAP * b : BCAP * b + len(blk)] = blk
        slot_cols[d] = cols_s
        valid = cols_s >= 0

        G = (node_adj[:, np.where(valid, cols_s, 0)] != 0).astype(np.float32)
        G[:, ~valid] = 0.0
        # rotate rows: local row r = absolute row (r + 1024d) mod N
        G = np.concatenate([G[lo:], G[:lo]], axis=0)
        lc = np.where(valid, cols_s - lo, -1)  # local split row (diag) per slot
        G[lc[valid], s_idx[valid]] = 0.0       # zero the diagonal
        block = G[base[None, :] + rows128[:, None], s_idx[None, :]]  # [128, CAP]
        lrow = base[None, :] + rows128[:, None]
        diagL = np.where(lrow < lc[None, :], block, 0.0)
        diagU = np.where(lrow > lc[None, :], block, 0.0)
        G[base[None, :] + rows128[:, None], s_idx[None, :]] = diagL
        # flat [128, 64*512]: adjf[p, 512j + s] = G[128j + p, s]
        adjf = np.ascontiguousarray(
            G.reshape(NT, P, CAP).transpose(1, 0, 2).reshape(P, NT * CAP).astype(FP8)
        )
        in_maps.append(
            {
                "adj": adjf,
                "diagu": np.ascontiguousarray(diagU.astype(FP8)),
                "wmat": _build_wmat(wside, d),
            }
        )

    mult_of = np.zeros(N, np.int64)
    mult_of[ucols] = mult
    over_loss = _host_cols_loss(outputs, targets, node_adj, overflow, mult_of)
    ctx = {"slot_cols": slot_cols, "mult_of": mult_of, "over_loss": over_loss}
    return in_maps, ctx


def _host_cols_loss(outputs, targets, node_adj, cols, mult_of):
    """Reference-exact loss contribution of a few columns (bucket overflow only)."""
    if not cols:
        return 0.0
    cols = np.asarray(cols, np.int64)
    out = np.asarray(outputs, np.float64).reshape(-1)
    pos = np.asarray(targets).reshape(-1) != 0
    A = node_adj[:, cols] != 0
    r = np.arange(N)[:, None]
    A = A & (r != cols[None, :])
    total = 0.0
    for mask in (A & (r < cols[None, :]), A & (r > cols[None, :])):
        cnt = mask.sum(axis=0)
        poscnt = (mask & pos[:, None]).sum(axis=0)
        sumexp = (mask * np.exp(out)[:, None]).sum(axis=0)
        poslogit = (mask * (pos * out)[:, None]).sum(axis=0)
        valid = (cnt > 0) & (poscnt == 1)
        contrib = np.where(
            valid,
            (np.log(np.maximum(sumexp, 1e-300)) - poslogit) / np.maximum(cnt, 1),
            0.0,
        )
        total += (contrib * mult_of[cols]).sum()
    return total


def _combine(stats_list, ctx):
    """Per-core stats [12, CAP] f32 -> scalar loss (f64 math)."""

    def side_contrib(x):
        cnt, poscnt = x[0], x[1]
        poslogit = x[2] + x[3]
        sumexp = x[4] + x[5]
        valid = (cnt > 0.5) & (np.abs(poscnt - 1.0) < 0.25)
        lse = np.log(np.where(valid, np.maximum(sumexp, 1e-300), 1.0))
        return np.where(valid, (lse - poslogit) / np.maximum(cnt, 1.0), 0.0)

    total = ctx["over_loss"]
    for d, s in enumerate(stats_list):
        x = np.asarray(s, np.float64)
        contrib = side_contrib(x[0:NW]) + side_contrib(x[NW:M])
        cols = ctx["slot_cols"][d]
        valid = cols >= 0
        total += (contrib[valid] * ctx["mult_of"][cols[valid]]).sum()
    return np.array(total, dtype=np.float32)


def _ensure_axon_hooks_stub():
    """bass_utils imports antenv.axon_hooks when tracing is requested via
    env; the module is absent on some images. Provide a no-op stub so the
    import never crashes (hook=None -> bass_utils skips tracing)."""
    import sys
    import types

    try:
        import antenv.axon_hooks  # noqa: F401
    except ImportError:
        mod = types.ModuleType("antenv.axon_hooks")
        state = {"hook": None}
        mod.set_axon_ntff_profile_hook = lambda h: state.__setitem__("hook", h)
        mod.get_axon_ntff_profile_hook = lambda: state["hook"]
        sys.modules["antenv.axon_hooks"] = mod


def _device_stats(in_maps):
    _ensure_axon_hooks_stub()
    from concourse.bass_utils import run_bass_kernel_spmd

    if "nc" not in _BASS_CACHE:
        _BASS_CACHE["nc"] = _build_bass()
    last_exc = None
    for attempt in range(4):
        try:
            res = run_bass_kernel_spmd(
                _BASS_CACHE["nc"], in_maps, core_ids=list(range(NCORES))
            )
            return [r["stats"] for r in res.results]
        except Exception as e:  # transient NRT/accelerator hiccups
            last_exc = e
            try:
                # a fresh PJRT client usually recovers a transiently
                # "unrecoverable" accelerator; mirrors a process restart
                import jax
                import jax.extend.backend as _jeb

                jax.clear_caches()
                _jeb.clear_backends()
            except Exception:
                pass
            import time

            time.sleep(2.0 * (attempt + 1))
    raise last_exc


def _sim_stats(in_maps):
    """Numpy emulation of the device kernel (same inputs), for logic validation."""
    outs = []
    for m in in_maps:
        adjf = m["adj"].astype(np.float32)
        diagu = m["diagu"].astype(np.float32)
        w = m["wmat"].reshape(P, NT + TPC, M).astype(np.float32)
        acc = np.zeros((M, CAP), np.float32)
        for j in range(NT):
            tile = adjf[:, j * CAP : (j + 1) * CAP]
            if j < TPC:
                c0 = BCAP * j
                acc[:, c0:] += w[:, j, :].T @ tile[:, c0:]
                acc[:, :c0] += w[:, NT + j, :].T @ tile[:, :c0]
                acc[:, c0 : c0 + BCAP] += w[:, NT + j, :].T @ diagu[:, c0 : c0 + BCAP]
            else:
                acc += w[:, j, :].T @ tile
        outs.append(acc)
    return outs


def kernel(outputs, targets, node_adj, idx_node, _simulate=False):
    in_maps, ctx = _prepare(outputs, targets, node_adj, idx_node)
    stats = _sim_stats(in_maps) if _simulate else _device_stats(in_maps)
    return _combine(stats, ctx)


# revision 5
# speedup vs baseline: 3.7559x; 1.1436x over previous
"""Trainium2 Bass kernel for nn_CELoss_4896262717859 (v3: fp8 DoubleRow).

For each query column c = idx_node[k] of a sparse adjacency matrix (diagonal
zeroed), a cross-entropy-style loss over the "lower" (r < c) and "upper"
(r > c) neighbor sets:

    contrib_side(c) = [cnt>0 and poscnt==1] * (log(sum_r m exp(out_r)) - poslogit) / cnt

All per-column quantities are sums  sum_r adj[r,c] * w[r]  for
w in {1, pos, pos*out, exp(out)} -> tensor-engine matvecs with a triangular
(L/U) split, computed ONLY for the distinct idx_node columns (~3218 of 8192),
then combined with multiplicities on the host (O(N+K)).

Sharding: core d handles the distinct query columns falling in column slab
[1024d, 1024(d+1)).  Within a slab, columns are bucketed by the 128-row block
containing their diagonal (the "mixed" block); each of the 8 buckets is padded
to a fixed 64 slots -> exactly 512 column slots per core, so ONE compiled
program (fixed matmul ranges) serves every core and any input.  Rows are
rotated by 1024d so the mixed blocks always land in local row-tiles 0..7.

Everything streams as fp8e4 (adjacency 0/1 exact; weights hi/mid/lo split
-> ~12 mantissa bits): 4.2 MB/core, and the 56 non-mixed row tiles run as 28
DoubleRow matmul pairs (2 fp8 MACs/cell/cycle).  The mixed 128-row block of
each column is pre-masked on the host: its lower part (rows < c) replaces the
block in the main slab (covered by the L matmul), its upper part goes to a
small separate diagu[128,512] operand (one extra 64-wide matmul per diagonal
tile).  No on-device casts or mask multiplies remain.

Any bucket overflow beyond 64 distinct columns (never happens for uniform
idx_node; p<1% per bucket) falls back to a tiny host-side computation for the
overflowed columns only.
"""

import numpy as np
import ml_dtypes

N = 8192
K = 4096
NCORES = 8
SLAB = N // NCORES        # 1024 columns per slab
P = 128                   # partition / tile edge
NT = N // P               # 64 row tiles
TPC = SLAB // P           # 8 diagonal (mixed) tiles per core
NW = 8                    # weights per side: {1, pos, pl_h, pl_m, pl_l, e_h, e_m, e_l}
M = 2 * NW                # 16 psum partitions (L half = 0:8, U half = 8:16)
VW = 16                   # weight-variant stride (cols); == M, and 16B for fp8
CAP = 512                 # column slots per core (one psum bank)
BCAP = CAP // TPC         # 64 slots per 128-row bucket
CHUNK_TILES = (2, 2, 4, 8, 8, 8, 8, 8, 8, 8)   # row tiles per DMA chunk

BF16 = ml_dtypes.bfloat16
FP8 = ml_dtypes.float8_e4m3fn

_BASS_CACHE = {}


def _build_bass():
    import concourse.tile as tile
    import concourse.mybir as mybir
    from concourse import bacc

    f8 = mybir.dt.float8e4
    # Bacc (not raw Bass): its compile() runs generate_event_semaphores,
    # which splits multi-sem waits — TRN2 instructions hold at most one.
    nc = bacc.Bacc("TRN2")
    adj = nc.dram_tensor("adj", [P, NT * CAP], f8, kind="ExternalInput")
    diagu = nc.dram_tensor("diagu", [P, CAP], f8, kind="ExternalInput")
    wmat = nc.dram_tensor("wmat", [P, (NT + TPC) * VW], f8, kind="ExternalInput")
    stats = nc.dram_tensor("stats", [M, CAP], mybir.dt.float32, kind="ExternalOutput")

    with tile.TileContext(nc) as tc:
        with (
            tc.tile_pool(name="singles", bufs=1) as singles,
            tc.tile_pool(name="psum", bufs=1, space="PSUM") as psum_pool,
        ):
            # weights first (every matmul needs them), then the adjacency
            # chunks in ascending-size order so the first matmuls start ASAP.
            # Issue alternates between the two HWDGE engines (sync / scalar)
            # so descriptor generation isn't serialized on one queue.
            wsb = singles.tile([P, NT + TPC, VW], f8)
            nc.sync.dma_start(out=wsb, in_=wmat[:, :])
            dsb = singles.tile([P, CAP], f8)
            nc.scalar.dma_start(out=dsb, in_=diagu[:, :])
            chunks = []   # (tile, first_tile, ntiles)
            t0 = 0
            for ci, nt in enumerate(CHUNK_TILES):
                t = singles.tile([P, nt, CAP], f8, name=f"ch{ci}")
                eng = nc.sync if ci % 2 == 0 else nc.scalar
                eng.dma_start(out=t, in_=adj[:, t0 * CAP : (t0 + nt) * CAP])
                chunks.append((t, t0, nt))
                t0 += nt
            assert t0 == NT

            acc = psum_pool.tile([M, CAP], mybir.dt.float32, name="acc")

            def wv(v, n=1):
                return wsb[:, v : v + n, :]

            def chunk_rhs(j, n=1):
                for t, t0, nt in chunks:
                    if t0 <= j and j + n <= t0 + nt:
                        return t[:, j - t0 : j - t0 + n, :]
                raise AssertionError(f"tile {j}+{n} spans chunks")

            for j in range(TPC):
                c0 = BCAP * j
                rhs = chunk_rhs(j)
                # rows < c for slots right of (and inside, pre-masked) the
                # mixed bucket -> L weights
                nc.tensor.matmul(
                    acc[:, c0:CAP], wv(j), rhs[:, :, c0:CAP],
                    start=(j == 0), stop=False, skip_group_check=True,
                )
                if j > 0:
                    # rows > c for slots left of the mixed bucket -> U
                    nc.tensor.matmul(
                        acc[:, 0:c0], wv(NT + j), rhs[:, :, 0:c0],
                        start=False, stop=False, skip_group_check=True,
                    )
                # upper part of the mixed bucket itself
                nc.tensor.matmul(
                    acc[:, c0 : c0 + BCAP], wv(NT + j), dsb[:, c0 : c0 + BCAP],
                    start=False, stop=False, skip_group_check=True,
                )
            for j in range(TPC, NT, 2):
                nc.tensor.matmul(
                    acc[:, :], wv(j, 2), chunk_rhs(j, 2),
                    start=False, stop=(j == NT - 2), skip_group_check=True,
                    perf_mode=mybir.MatmulPerfMode.DoubleRow,
                )

            out_sb = singles.tile([M, CAP], mybir.dt.float32)
            half = CAP // 2
            nc.scalar.copy(out_sb[:, 0:half], acc[:, 0:half])
            nc.vector.tensor_copy(out_sb[:, half:], acc[:, half:])
            nc.sync.dma_start(out=stats[:, :], in_=out_sb)

    nc.compile()
    return nc


def _split_fp8(v, terms=3):
    """Split f64 vector into `terms` fp8 values summing to ~v (12 mantissa bits)."""
    out = []
    r = np.asarray(v, np.float64)
    for _ in range(terms):
        t = r.astype(FP8)
        out.append(t)
        r = r - t.astype(np.float64)
    return out


def _make_wside(outputs, targets):
    """Per-row weight table [N, 8] fp8."""
    out = np.asarray(outputs, np.float64).reshape(-1)
    pos = (np.asarray(targets).reshape(-1) != 0).astype(np.float64)
    cols = [np.ones(N, FP8), pos.astype(FP8)]
    cols += _split_fp8(pos * out)
    cols += _split_fp8(np.exp(out))
    return np.stack(cols, axis=1).astype(FP8)  # [N, 8]


def _build_wmat(wside, core):
    """Per-core weight variants [128, (64+8)*16] fp8.

    Variant j (j<64): weights for local row tile j (absolute tile (8*core+j)%64).
      j < 8  -> L-only variant (diag tiles; U-only twin stored at 64+j)
      j >= 8 -> single variant, L or U half per the tile's position vs the slab
    """
    w = np.zeros((P, NT + TPC, VW), dtype=FP8)
    for j in range(NT):
        t = (TPC * core + j) % NT
        rows = wside[t * P : (t + 1) * P, :]
        if j < TPC:
            w[:, j, 0:NW] = rows
            w[:, NT + j, NW:M] = rows
        elif j < NT - TPC * core:
            w[:, j, NW:M] = rows  # rows above slab columns -> U
        else:
            w[:, j, 0:NW] = rows  # wrapped rows below slab columns -> L
    return np.ascontiguousarray(w.reshape(P, (NT + TPC) * VW))


def _prepare(outputs, targets, node_adj, idx_node):
    """Build per-core in_maps + combine context (slot->column map, multiplicities,
    host-computed contribution of any bucket-overflow columns)."""
    node_adj = np.asarray(node_adj)
    idx = np.asarray(idx_node).reshape(-1).astype(np.int64)
    ucols, mult = np.unique(idx, return_counts=True)
    wside = _make_wside(outputs, targets)

    in_maps = []
    slot_cols = np.full((NCORES, CAP), -1, np.int64)
    overflow = []
    rows128 = np.arange(P)
    s_idx = np.arange(CAP)
    base = P * (s_idx // BCAP)  # first local row of each slot's mixed block

    for d in range(NCORES):
        lo = SLAB * d
        uc = ucols[(ucols >= lo) & (ucols < lo + SLAB)]
        cols_s = np.full(CAP, -1, np.int64)
        for b in range(TPC):
            blk = uc[(uc - lo) // P == b]
            if len(blk) > BCAP:
                overflow.extend(blk[BCAP:].tolist())
                blk = blk[:BCAP]
            cols_s[BCAP * b : BCAP * b + len(blk)] = blk
        slot_cols[d] = cols_s
        valid = cols_s >= 0

        G = (node_adj[:, np.where(valid, cols_s, 0)] != 0).astype(np.float32)
        G[:, ~valid] = 0.0
        # rotate rows: local row r = absolute row (r + 1024d) mod N
        G = np.concatenate([G[lo:], G[:lo]], axis=0)
        lc = np.where(valid, cols_s - lo, -1)  # local split row (diag) per slot
        G[lc[valid], s_idx[valid]] = 0.0       # zero the diagonal
        block = G[base[None, :] + rows128[:, None], s_idx[None, :]]  # [128, CAP]
        lrow = base[None, :] + rows128[:, None]
        diagL = np.where(lrow < lc[None, :], block, 0.0)
        diagU = np.where(lrow > lc[None, :], block, 0.0)
        G[base[None, :] + rows128[:, None], s_idx[None, :]] = diagL
        # flat [128, 64*512]: adjf[p, 512j + s] = G[128j + p, s]
        adjf = np.ascontiguousarray(
            G.reshape(NT, P, CAP).transpose(1, 0, 2).reshape(P, NT * CAP).astype(FP8)
        )
        in_maps.append(
            {
                "adj": adjf,
                "diagu": np.ascontiguousarray(diagU.astype(FP8)),
                "wmat": _build_wmat(wside, d),
            }
        )

    mult_of = np.zeros(N, np.int64)
    mult_of[ucols] = mult
    over_loss = _host_cols_loss(outputs, targets, node_adj, overflow, mult_of)
    ctx = {"slot_cols": slot_cols, "mult_of": mult_of, "over_loss": over_loss}
    return in_maps, ctx


def _host_cols_loss(outputs, targets, node_adj, cols, mult_of):
    """Reference-exact loss contribution of a few columns (bucket overflow only)."""
    if not cols:
        return 0.0
    cols = np.asarray(cols, np.int64)
    out = np.asarray(outputs, np.float64).reshape(-1)
    pos = np.asarray(targets).reshape(-1) != 0
    A = node_adj[:, cols] != 0
    r = np.arange(N)[:, None]
    A = A & (r != cols[None, :])
    total = 0.0
    for mask in (A & (r < cols[None, :]), A & (r > cols[None, :])):
        cnt = mask.sum(axis=0)
        poscnt = (mask & pos[:, None]).sum(axis=0)
        sumexp = (mask * np.exp(out)[:, None]).sum(axis=0)
        poslogit = (mask * (pos * out)[:, None]).sum(axis=0)
        valid = (cnt > 0) & (poscnt == 1)
        contrib = np.where(
            valid,
            (np.log(np.maximum(sumexp, 1e-300)) - poslogit) / np.maximum(cnt, 1),
            0.0,
        )
        total += (contrib * mult_of[cols]).sum()
    return total


def _combine(stats_list, ctx):
    """Per-core stats [16, CAP] f32 -> scalar loss (f64 math)."""

    def side_contrib(x):
        cnt, poscnt = x[0], x[1]
        poslogit = x[2] + x[3] + x[4]
        sumexp = x[5] + x[6] + x[7]
        valid = (cnt > 0.5) & (np.abs(poscnt - 1.0) < 0.25)
        lse = np.log(np.where(valid, np.maximum(sumexp, 1e-300), 1.0))
        return np.where(valid, (lse - poslogit) / np.maximum(cnt, 1.0), 0.0)

    total = ctx["over_loss"]
    for d, s in enumerate(stats_list):
        x = np.asarray(s, np.float64)
        contrib = side_contrib(x[0:NW]) + side_contrib(x[NW:M])
        cols = ctx["slot_cols"][d]
        valid = cols >= 0
        total += (contrib[valid] * ctx["mult_of"][cols[valid]]).sum()
    return np.array(total, dtype=np.float32)


def _ensure_axon_hooks_stub():
    """bass_utils imports antenv.axon_hooks when tracing is requested via
    env; the module is absent on some images. Provide a no-op stub so the
    import never crashes (hook=None -> bass_utils skips tracing)."""
    import sys
    import types

    try:
        import antenv.axon_hooks  # noqa: F401
    except ImportError:
        mod = types.ModuleType("antenv.axon_hooks")
        state = {"hook": None}
        mod.set_axon_ntff_profile_hook = lambda h: state.__setitem__("hook", h)
        mod.get_axon_ntff_profile_hook = lambda: state["hook"]
        sys.modules["antenv.axon_hooks"] = mod


def _device_stats(in_maps):
    _ensure_axon_hooks_stub()
    from concourse.bass_utils import run_bass_kernel_spmd

    if "nc" not in _BASS_CACHE:
        _BASS_CACHE["nc"] = _build_bass()
    last_exc = None
    for attempt in range(4):
        try:
            res = run_bass_kernel_spmd(
                _BASS_CACHE["nc"], in_maps, core_ids=list(range(NCORES))
            )
            return [r["stats"] for r in res.results]
        except Exception as e:  # transient NRT/accelerator hiccups
            last_exc = e
            try:
                # a fresh PJRT client usually recovers a transiently
                # "unrecoverable" accelerator; mirrors a process restart
                import jax
                import jax.extend.backend as _jeb

                jax.clear_caches()
                _jeb.clear_backends()
            except Exception:
                pass
            import time

            time.sleep(2.0 * (attempt + 1))
    raise last_exc


def _sim_stats(in_maps):
    """Numpy emulation of the device kernel (same inputs), for logic validation."""
    outs = []
    for m in in_maps:
        adjf = m["adj"].astype(np.float32)
        diagu = m["diagu"].astype(np.float32)
        w = m["wmat"].reshape(P, NT + TPC, VW).astype(np.float32)
        acc = np.zeros((M, CAP), np.float32)
        for j in range(NT):
            tile = adjf[:, j * CAP : (j + 1) * CAP]
            if j < TPC:
                c0 = BCAP * j
                acc[:, c0:] += w[:, j, :M].T @ tile[:, c0:]
                acc[:, :c0] += w[:, NT + j, :M].T @ tile[:, :c0]
                acc[:, c0 : c0 + BCAP] += w[:, NT + j, :M].T @ diagu[:, c0 : c0 + BCAP]
            else:
                acc += w[:, j, :M].T @ tile
        outs.append(acc)
    return outs


def kernel(outputs, targets, node_adj, idx_node, _simulate=False):
    in_maps, ctx = _prepare(outputs, targets, node_adj, idx_node)
    stats = _sim_stats(in_maps) if _simulate else _device_stats(in_maps)
    return _combine(stats, ctx)


# revision 9
# speedup vs baseline: 3.8196x; 1.0170x over previous
"""Trainium2 Bass kernel for nn_CELoss_4896262717859 (v3: fp8 DoubleRow).

For each query column c = idx_node[k] of a sparse adjacency matrix (diagonal
zeroed), a cross-entropy-style loss over the "lower" (r < c) and "upper"
(r > c) neighbor sets:

    contrib_side(c) = [cnt>0 and poscnt==1] * (log(sum_r m exp(out_r)) - poslogit) / cnt

All per-column quantities are sums  sum_r adj[r,c] * w[r]  for
w in {1, pos, pos*out, exp(out)} -> tensor-engine matvecs with a triangular
(L/U) split, computed ONLY for the distinct idx_node columns (~3218 of 8192),
then combined with multiplicities on the host (O(N+K)).

Sharding: core d handles the distinct query columns falling in column slab
[1024d, 1024(d+1)).  Within a slab, columns are bucketed by the 128-row block
containing their diagonal (the "mixed" block); each of the 8 buckets is padded
to a fixed 64 slots -> exactly 512 column slots per core, so ONE compiled
program (fixed matmul ranges) serves every core and any input.  Rows are
rotated by 1024d so the mixed blocks always land in local row-tiles 0..7.

Everything streams as fp8e4 (adjacency 0/1 exact; weights hi/mid/lo split
-> ~12 mantissa bits): 4.2 MB/core, and the 56 non-mixed row tiles run as 28
DoubleRow matmul pairs (2 fp8 MACs/cell/cycle).  The mixed 128-row block of
each column is pre-masked on the host: its lower part (rows < c) replaces the
block in the main slab (covered by the L matmul), its upper part goes to a
small separate diagu[128,512] operand (one extra 64-wide matmul per diagonal
tile).  No on-device casts or mask multiplies remain.

Any bucket overflow beyond 64 distinct columns (never happens for uniform
idx_node; p<1% per bucket) falls back to a tiny host-side computation for the
overflowed columns only.
"""

import numpy as np
import ml_dtypes

N = 8192
K = 4096
NCORES = 8
SLAB = N // NCORES        # 1024 columns per slab
P = 128                   # partition / tile edge
NT = N // P               # 64 row tiles
TPC = SLAB // P           # 8 diagonal (mixed) tiles per core
NW = 8                    # weights per side: {1, pos, pl_h, pl_m, pl_l, e_h, e_m, e_l}
M = 2 * NW                # 16 psum partitions (L half = 0:8, U half = 8:16)
VW = 16                   # weight-variant stride (cols); == M, and 16B for fp8
CAP = 512                 # column slots per core (one psum bank)
BCAP = CAP // TPC         # 64 slots per 128-row bucket
CHUNK_TILES = (4, 4, 8, 12, 12, 12, 12)   # row tiles per DMA chunk
NAUX = (NT + TPC) + TPC * (BCAP // VW)    # aux columns of 16: 72 wmat + 32 diagu

BF16 = ml_dtypes.bfloat16
FP8 = ml_dtypes.float8_e4m3fn

_BASS_CACHE = {}


def _build_bass():
    import concourse.tile as tile
    import concourse.mybir as mybir
    from concourse import bacc

    f8 = mybir.dt.float8e4
    DR = mybir.MatmulPerfMode.DoubleRow
    # Bacc (not raw Bass): its compile() runs generate_event_semaphores,
    # which splits multi-sem waits — TRN2 instructions hold at most one.
    nc = bacc.Bacc("TRN2")
    adj = nc.dram_tensor("adj", [P, NT * CAP], f8, kind="ExternalInput")
    # aux = wmat variants (72x16) ++ diagu (512 = 32x16)
    aux = nc.dram_tensor("aux", [P, NAUX * VW], f8, kind="ExternalInput")
    stats = nc.dram_tensor("stats", [M, CAP], mybir.dt.float32, kind="ExternalOutput")

    with tile.TileContext(nc) as tc:
        with (
            tc.tile_pool(name="singles", bufs=1) as singles,
            tc.tile_pool(name="psum", bufs=1, space="PSUM") as psum_pool,
        ):
            # aux (weights+diagu) first: every matmul needs it.  7 input DMAs
            # + 2 output DMAs: at most one benign reuse of the 8 HWDGE sem
            # lanes (the reused lane's DMA completed long before).  Issue
            # alternates between the two HWDGE engines (sync / scalar) so
            # descriptor generation and ring draining run in parallel.
            asb = singles.tile([P, NAUX, VW], f8)
            nc.sync.dma_start(out=asb, in_=aux[:, :])
            chunks = []   # (tile, first_tile, ntiles)
            t0 = 0
            for ci, nt in enumerate(CHUNK_TILES):
                t = singles.tile([P, nt, CAP], f8, name=f"ch{ci}")
                eng = nc.scalar if ci % 2 == 0 else nc.sync
                eng.dma_start(out=t, in_=adj[:, t0 * CAP : (t0 + nt) * CAP])
                chunks.append((t, t0, nt))
                t0 += nt
            assert t0 == NT

            acc = psum_pool.tile([M, CAP], mybir.dt.float32, name="acc")

            def wv(v, n=1):
                return asb[:, v : v + n, :]

            def du(b, n=1):  # diagu strip for mixed block b: 64 cols = 4 aux vars
                q = BCAP // VW
                return asb[:, NT + TPC + q * b : NT + TPC + q * (b + n), :]

            def chunk_rhs(j, n=1):
                for t, t0, nt in chunks:
                    if t0 <= j and j + n <= t0 + nt:
                        return t[:, j - t0 : j - t0 + n, :]
                raise AssertionError(f"tile {j}+{n} spans chunks")

            def mm(out_ap, w, rhs, **kw):
                nc.tensor.matmul(out_ap, w, rhs, skip_group_check=True,
                                 start=kw.pop("start", False),
                                 stop=kw.pop("stop", False), **kw)

            # Diagonal (mixed) tiles, DoubleRow-paired on their shared spans.
            # Tile j's L span is [64j, 512) (its own mixed bucket holds
            # host-pre-masked lower data), U span is [0, 64j); the mixed
            # bucket's upper part comes from the diagu strips.
            for j in range(0, TPC, 2):
                a, b = BCAP * j, BCAP * (j + 1)
                rhs2 = chunk_rhs(j, 2)
                # shared L span of the pair
                mm(acc[:, b:CAP], wv(j, 2), rhs2[:, :, b:CAP],
                   start=(j == 0), perf_mode=DR)
                # tile j's extra L strip (its own bucket)
                mm(acc[:, a:b], wv(j), chunk_rhs(j)[:, :, a:b])
                if j > 0:
                    # shared U span of the pair
                    mm(acc[:, 0:a], wv(NT + j, 2), rhs2[:, :, 0:a], perf_mode=DR)
                # tile j+1's extra U strip (tile j's bucket columns)
                mm(acc[:, a:b], wv(NT + j + 1), chunk_rhs(j + 1)[:, :, a:b])
                # upper parts of the mixed buckets themselves
                mm(acc[:, a:b], wv(NT + j), du(j))
                mm(acc[:, b : b + BCAP], wv(NT + j + 1), du(j + 1))
            for j in range(TPC, NT, 2):
                mm(acc[:, :], wv(j, 2), chunk_rhs(j, 2),
                   stop=(j == NT - 2), perf_mode=DR)

            out_sb = singles.tile([M, CAP], mybir.dt.float32)
            half = CAP // 2
            nc.vector.tensor_copy(out_sb[:, half:], acc[:, half:])
            nc.scalar.copy(out_sb[:, 0:half], acc[:, 0:half])
            nc.sync.dma_start(out=stats[:, 0:half], in_=out_sb[:, 0:half])
            nc.scalar.dma_start(out=stats[:, half:], in_=out_sb[:, half:])

    nc.compile()
    return nc


def _split_fp8(v, terms=3):
    """Split f64 vector into `terms` fp8 values summing to ~v (12 mantissa bits)."""
    out = []
    r = np.asarray(v, np.float64)
    for _ in range(terms):
        t = r.astype(FP8)
        out.append(t)
        r = r - t.astype(np.float64)
    return out


def _make_wside(outputs, targets):
    """Per-row weight table [N, 8] fp8."""
    out = np.asarray(outputs, np.float64).reshape(-1)
    pos = (np.asarray(targets).reshape(-1) != 0).astype(np.float64)
    cols = [np.ones(N, FP8), pos.astype(FP8)]
    cols += _split_fp8(pos * out)
    cols += _split_fp8(np.exp(out))
    return np.stack(cols, axis=1).astype(FP8)  # [N, 8]


def _build_wmat(wside, core):
    """Per-core weight variants [128, (64+8)*16] fp8.

    Variant j (j<64): weights for local row tile j (absolute tile (8*core+j)%64).
      j < 8  -> L-only variant (diag tiles; U-only twin stored at 64+j)
      j >= 8 -> single variant, L or U half per the tile's position vs the slab
    """
    w = np.zeros((P, NT + TPC, VW), dtype=FP8)
    for j in range(NT):
        t = (TPC * core + j) % NT
        rows = wside[t * P : (t + 1) * P, :]
        if j < TPC:
            w[:, j, 0:NW] = rows
            w[:, NT + j, NW:M] = rows
        elif j < NT - TPC * core:
            w[:, j, NW:M] = rows  # rows above slab columns -> U
        else:
            w[:, j, 0:NW] = rows  # wrapped rows below slab columns -> L
    return np.ascontiguousarray(w.reshape(P, (NT + TPC) * VW))


def _prepare(outputs, targets, node_adj, idx_node):
    """Build per-core in_maps + combine context (slot->column map, multiplicities,
    host-computed contribution of any bucket-overflow columns)."""
    node_adj = np.asarray(node_adj)
    idx = np.asarray(idx_node).reshape(-1).astype(np.int64)
    ucols, mult = np.unique(idx, return_counts=True)
    wside = _make_wside(outputs, targets)

    in_maps = []
    slot_cols = np.full((NCORES, CAP), -1, np.int64)
    overflow = []
    rows128 = np.arange(P)
    s_idx = np.arange(CAP)
    base = P * (s_idx // BCAP)  # first local row of each slot's mixed block

    for d in range(NCORES):
        lo = SLAB * d
        uc = ucols[(ucols >= lo) & (ucols < lo + SLAB)]
        cols_s = np.full(CAP, -1, np.int64)
        for b in range(TPC):
            blk = uc[(uc - lo) // P == b]
            if len(blk) > BCAP:
                overflow.extend(blk[BCAP:].tolist())
                blk = blk[:BCAP]
            cols_s[BCAP * b : BCAP * b + len(blk)] = blk
        slot_cols[d] = cols_s
        valid = cols_s >= 0

        G = (node_adj[:, np.where(valid, cols_s, 0)] != 0).astype(np.float32)
        G[:, ~valid] = 0.0
        # rotate rows: local row r = absolute row (r + 1024d) mod N
        G = np.concatenate([G[lo:], G[:lo]], axis=0)
        lc = np.where(valid, cols_s - lo, -1)  # local split row (diag) per slot
        G[lc[valid], s_idx[valid]] = 0.0       # zero the diagonal
        block = G[base[None, :] + rows128[:, None], s_idx[None, :]]  # [128, CAP]
        lrow = base[None, :] + rows128[:, None]
        diagL = np.where(lrow < lc[None, :], block, 0.0)
        diagU = np.where(lrow > lc[None, :], block, 0.0)
        G[base[None, :] + rows128[:, None], s_idx[None, :]] = diagL
        # flat [128, 64*512]: adjf[p, 512j + s] = G[128j + p, s]
        adjf = np.ascontiguousarray(
            G.reshape(NT, P, CAP).transpose(1, 0, 2).reshape(P, NT * CAP).astype(FP8)
        )
        in_maps.append(
            {
                "adj": adjf,
                "aux": np.ascontiguousarray(
                    np.concatenate(
                        [_build_wmat(wside, d), diagU.astype(FP8)], axis=1
                    )
                ),
            }
        )

    mult_of = np.zeros(N, np.int64)
    mult_of[ucols] = mult
    over_loss = _host_cols_loss(outputs, targets, node_adj, overflow, mult_of)
    ctx = {"slot_cols": slot_cols, "mult_of": mult_of, "over_loss": over_loss}
    return in_maps, ctx


def _host_cols_loss(outputs, targets, node_adj, cols, mult_of):
    """Reference-exact loss contribution of a few columns (bucket overflow only)."""
    if not cols:
        return 0.0
    cols = np.asarray(cols, np.int64)
    out = np.asarray(outputs, np.float64).reshape(-1)
    pos = np.asarray(targets).reshape(-1) != 0
    A = node_adj[:, cols] != 0
    r = np.arange(N)[:, None]
    A = A & (r != cols[None, :])
    total = 0.0
    for mask in (A & (r < cols[None, :]), A & (r > cols[None, :])):
        cnt = mask.sum(axis=0)
        poscnt = (mask & pos[:, None]).sum(axis=0)
        sumexp = (mask * np.exp(out)[:, None]).sum(axis=0)
        poslogit = (mask * (pos * out)[:, None]).sum(axis=0)
        valid = (cnt > 0) & (poscnt == 1)
        contrib = np.where(
            valid,
            (np.log(np.maximum(sumexp, 1e-300)) - poslogit) / np.maximum(cnt, 1),
            0.0,
        )
        total += (contrib * mult_of[cols]).sum()
    return total


def _combine(stats_list, ctx):
    """Per-core stats [16, CAP] f32 -> scalar loss (f64 math)."""

    def side_contrib(x):
        cnt, poscnt = x[0], x[1]
        poslogit = x[2] + x[3] + x[4]
        sumexp = x[5] + x[6] + x[7]
        valid = (cnt > 0.5) & (np.abs(poscnt - 1.0) < 0.25)
        lse = np.log(np.where(valid, np.maximum(sumexp, 1e-300), 1.0))
        return np.where(valid, (lse - poslogit) / np.maximum(cnt, 1.0), 0.0)

    total = ctx["over_loss"]
    for d, s in enumerate(stats_list):
        x = np.asarray(s, np.float64)
        contrib = side_contrib(x[0:NW]) + side_contrib(x[NW:M])
        cols = ctx["slot_cols"][d]
        valid = cols >= 0
        total += (contrib[valid] * ctx["mult_of"][cols[valid]]).sum()
    return np.array(total, dtype=np.float32)


def _ensure_axon_hooks_stub():
    """bass_utils imports antenv.axon_hooks when tracing is requested via
    env; the module is absent on some images. Provide a no-op stub so the
    import never crashes (hook=None -> bass_utils skips tracing)."""
    import sys
    import types

    try:
        import antenv.axon_hooks  # noqa: F401
    except ImportError:
        mod = types.ModuleType("antenv.axon_hooks")
        state = {"hook": None}
        mod.set_axon_ntff_profile_hook = lambda h: state.__setitem__("hook", h)
        mod.get_axon_ntff_profile_hook = lambda: state["hook"]
        sys.modules["antenv.axon_hooks"] = mod


def _device_stats(in_maps):
    _ensure_axon_hooks_stub()
    from concourse.bass_utils import run_bass_kernel_spmd

    if "nc" not in _BASS_CACHE:
        _BASS_CACHE["nc"] = _build_bass()
    last_exc = None
    for attempt in range(4):
        try:
            res = run_bass_kernel_spmd(
                _BASS_CACHE["nc"], in_maps, core_ids=list(range(NCORES))
            )
            return [r["stats"] for r in res.results]
        except Exception as e:  # transient NRT/accelerator hiccups
            last_exc = e
            try:
                # a fresh PJRT client usually recovers a transiently
                # "unrecoverable" accelerator; mirrors a process restart
                import jax
                import jax.extend.backend as _jeb

                jax.clear_caches()
                _jeb.clear_backends()
            except Exception:
                pass
            import time

            time.sleep(2.0 * (attempt + 1))
    raise last_exc


def _sim_stats(in_maps):
    """Numpy emulation of the device kernel (same inputs), for logic validation."""
    outs = []
    for m in in_maps:
        adjf = m["adj"].astype(np.float32)
        diagu = m["aux"][:, (NT + TPC) * VW :].astype(np.float32)
        w = m["aux"][:, : (NT + TPC) * VW].reshape(P, NT + TPC, VW).astype(np.float32)
        acc = np.zeros((M, CAP), np.float32)
        for j in range(NT):
            tile = adjf[:, j * CAP : (j + 1) * CAP]
            if j < TPC:
                c0 = BCAP * j
                acc[:, c0:] += w[:, j, :M].T @ tile[:, c0:]
                acc[:, :c0] += w[:, NT + j, :M].T @ tile[:, :c0]
                acc[:, c0 : c0 + BCAP] += w[:, NT + j, :M].T @ diagu[:, c0 : c0 + BCAP]
            else:
                acc += w[:, j, :M].T @ tile
        outs.append(acc)
    return outs


def kernel(outputs, targets, node_adj, idx_node, _simulate=False):
    in_maps, ctx = _prepare(outputs, targets, node_adj, idx_node)
    stats = _sim_stats(in_maps) if _simulate else _device_stats(in_maps)
    return _combine(stats, ctx)


# revision 15
# speedup vs baseline: 3.8914x; 1.0188x over previous
"""Trainium2 Bass kernel for nn_CELoss_4896262717859 (v3: fp8 DoubleRow).

For each query column c = idx_node[k] of a sparse adjacency matrix (diagonal
zeroed), a cross-entropy-style loss over the "lower" (r < c) and "upper"
(r > c) neighbor sets:

    contrib_side(c) = [cnt>0 and poscnt==1] * (log(sum_r m exp(out_r)) - poslogit) / cnt

All per-column quantities are sums  sum_r adj[r,c] * w[r]  for
w in {1, pos, pos*out, exp(out)} -> tensor-engine matvecs with a triangular
(L/U) split, computed ONLY for the distinct idx_node columns (~3218 of 8192),
then combined with multiplicities on the host (O(N+K)).

Sharding: core d handles the distinct query columns falling in column slab
[1024d, 1024(d+1)).  Within a slab, columns are bucketed by the 128-row block
containing their diagonal (the "mixed" block); each of the 8 buckets is padded
to a fixed 64 slots -> exactly 512 column slots per core, so ONE compiled
program (fixed matmul ranges) serves every core and any input.  Rows are
rotated by 1024d so the mixed blocks always land in local row-tiles 0..7.

Everything streams as fp8e4 (adjacency 0/1 exact; weights hi/mid/lo split
-> ~12 mantissa bits): 4.2 MB/core, and the 56 non-mixed row tiles run as 28
DoubleRow matmul pairs (2 fp8 MACs/cell/cycle).  The mixed 128-row block of
each column is pre-masked on the host: its lower part (rows < c) replaces the
block in the main slab (covered by the L matmul), its upper part goes to a
small separate diagu[128,512] operand (one extra 64-wide matmul per diagonal
tile).  No on-device casts or mask multiplies remain.

Any bucket overflow beyond 64 distinct columns (never happens for uniform
idx_node; p<1% per bucket) falls back to a tiny host-side computation for the
overflowed columns only.
"""

import numpy as np
import ml_dtypes

N = 8192
K = 4096
NCORES = 8
SLAB = N // NCORES        # 1024 columns per slab
P = 128                   # partition / tile edge
NT = N // P               # 64 row tiles
TPC = SLAB // P           # 8 diagonal (mixed) tiles per core
NW = 8                    # weights per side: {1, pos, pl_h, pl_m, pl_l, e_h, e_m, e_l}
M = 2 * NW                # 16 psum partitions (L half = 0:8, U half = 8:16)
VW = 16                   # weight-variant stride (cols); == M, and 16B for fp8
CAP = 448                 # column slots per core (one psum bank)
BCAP = CAP // TPC         # 56 slots per 128-row bucket
CHUNK_TILES = (2, 2, 4, 8, 8, 10, 10, 10, 10)   # row tiles per DMA chunk

BF16 = ml_dtypes.bfloat16
FP8 = ml_dtypes.float8_e4m3fn

_BASS_CACHE = {}


def _build_bass():
    import concourse.tile as tile
    import concourse.mybir as mybir
    from concourse import bacc

    f8 = mybir.dt.float8e4
    DR = mybir.MatmulPerfMode.DoubleRow
    # Bacc (not raw Bass): its compile() runs generate_event_semaphores,
    # which splits multi-sem waits — TRN2 instructions hold at most one.
    nc = bacc.Bacc("TRN2")
    adj = nc.dram_tensor("adj", [P, NT * CAP], f8, kind="ExternalInput")
    wmat = nc.dram_tensor("wmat", [P, (NT + TPC) * VW], f8, kind="ExternalInput")
    diagu = nc.dram_tensor("diagu", [P, CAP], f8, kind="ExternalInput")
    stats = nc.dram_tensor("stats", [M, CAP], mybir.dt.float32, kind="ExternalOutput")

    with tile.TileContext(nc) as tc:
        with (
            tc.tile_pool(name="singles", bufs=1) as singles,
            tc.tile_pool(name="psum", bufs=1, space="PSUM") as psum_pool,
        ):
            # weights first: every matmul needs them.  DMAs round-robin over
            # THREE descriptor rings (sync HWDGE / scalar HWDGE / gpsimd
            # SWDGE) so descriptor generation and ring draining overlap; the
            # leading chunks are small so the first matmuls start early.
            asb = singles.tile([P, NT + TPC, VW], f8)
            nc.sync.dma_start(out=asb, in_=wmat[:, :])
            dsb = singles.tile([P, CAP], f8)
            nc.scalar.dma_start(out=dsb, in_=diagu[:, :])
            engs = (nc.gpsimd, nc.sync, nc.scalar)
            chunks = []   # (tile, first_tile, ntiles)
            t0 = 0
            for ci, nt in enumerate(CHUNK_TILES):
                t = singles.tile([P, nt, CAP], f8, name=f"ch{ci}")
                engs[ci % 3].dma_start(out=t, in_=adj[:, t0 * CAP : (t0 + nt) * CAP])
                chunks.append((t, t0, nt))
                t0 += nt
            assert t0 == NT

            acc = psum_pool.tile([M, CAP], mybir.dt.float32, name="acc")

            def wv(v, n=1):
                return asb[:, v : v + n, :]

            def du(b):  # diagu strip for mixed block b
                return dsb[:, BCAP * b : BCAP * (b + 1)]

            def chunk_rhs(j, n=1):
                for t, t0, nt in chunks:
                    if t0 <= j and j + n <= t0 + nt:
                        return t[:, j - t0 : j - t0 + n, :]
                raise AssertionError(f"tile {j}+{n} spans chunks")

            def mm(out_ap, w, rhs, **kw):
                nc.tensor.matmul(out_ap, w, rhs, skip_group_check=True,
                                 start=kw.pop("start", False),
                                 stop=kw.pop("stop", False), **kw)

            # Diagonal (mixed) tiles, DoubleRow-paired on their shared spans.
            # Tile j's L span is [64j, 512) (its own mixed bucket holds
            # host-pre-masked lower data), U span is [0, 64j); the mixed
            # bucket's upper part comes from the diagu strips.
            for j in range(0, TPC, 2):
                a, b = BCAP * j, BCAP * (j + 1)
                rhs2 = chunk_rhs(j, 2)
                # shared L span of the pair
                mm(acc[:, b:CAP], wv(j, 2), rhs2[:, :, b:CAP],
                   start=(j == 0), perf_mode=DR)
                # tile j's extra L strip (its own bucket)
                mm(acc[:, a:b], wv(j), chunk_rhs(j)[:, :, a:b])
                if j > 0:
                    # shared U span of the pair
                    mm(acc[:, 0:a], wv(NT + j, 2), rhs2[:, :, 0:a], perf_mode=DR)
                # tile j+1's extra U strip (tile j's bucket columns)
                mm(acc[:, a:b], wv(NT + j + 1), chunk_rhs(j + 1)[:, :, a:b])
                # upper parts of the mixed buckets themselves
                mm(acc[:, a:b], wv(NT + j), du(j))
                mm(acc[:, b : b + BCAP], wv(NT + j + 1), du(j + 1))
            for j in range(TPC, NT, 2):
                mm(acc[:, :], wv(j, 2), chunk_rhs(j, 2),
                   stop=(j == NT - 2), perf_mode=DR)

            out_sb = singles.tile([M, CAP], mybir.dt.float32)
            half = CAP // 2
            nc.vector.tensor_copy(out_sb[:, half:], acc[:, half:])
            nc.scalar.copy(out_sb[:, 0:half], acc[:, 0:half])
            nc.sync.dma_start(out=stats[:, 0:half], in_=out_sb[:, 0:half])
            nc.scalar.dma_start(out=stats[:, half:], in_=out_sb[:, half:])

    nc.compile()
    return nc


def _split_fp8(v, terms=3):
    """Split f64 vector into `terms` fp8 values summing to ~v (12 mantissa bits)."""
    out = []
    r = np.asarray(v, np.float64)
    for _ in range(terms):
        t = r.astype(FP8)
        out.append(t)
        r = r - t.astype(np.float64)
    return out


def _make_wside(outputs, targets):
    """Per-row weight table [N, 8] fp8."""
    out = np.asarray(outputs, np.float64).reshape(-1)
    pos = (np.asarray(targets).reshape(-1) != 0).astype(np.float64)
    cols = [np.ones(N, FP8), pos.astype(FP8)]
    cols += _split_fp8(pos * out)
    cols += _split_fp8(np.exp(out))
    return np.stack(cols, axis=1).astype(FP8)  # [N, 8]


def _build_wmat(wside, core):
    """Per-core weight variants [128, (64+8)*16] fp8.

    Variant j (j<64): weights for local row tile j (absolute tile (8*core+j)%64).
      j < 8  -> L-only variant (diag tiles; U-only twin stored at 64+j)
      j >= 8 -> single variant, L or U half per the tile's position vs the slab
    """
    w = np.zeros((P, NT + TPC, VW), dtype=FP8)
    for j in range(NT):
        t = (TPC * core + j) % NT
        rows = wside[t * P : (t + 1) * P, :]
        if j < TPC:
            w[:, j, 0:NW] = rows
            w[:, NT + j, NW:M] = rows
        elif j < NT - TPC * core:
            w[:, j, NW:M] = rows  # rows above slab columns -> U
        else:
            w[:, j, 0:NW] = rows  # wrapped rows below slab columns -> L
    return np.ascontiguousarray(w.reshape(P, (NT + TPC) * VW))


def _prepare(outputs, targets, node_adj, idx_node):
    """Build per-core in_maps + combine context (slot->column map, multiplicities,
    host-computed contribution of any bucket-overflow columns)."""
    node_adj = np.asarray(node_adj)
    idx = np.asarray(idx_node).reshape(-1).astype(np.int64)
    ucols, mult = np.unique(idx, return_counts=True)
    wside = _make_wside(outputs, targets)

    in_maps = []
    slot_cols = np.full((NCORES, CAP), -1, np.int64)
    overflow = []
    rows128 = np.arange(P)
    s_idx = np.arange(CAP)
    base = P * (s_idx // BCAP)  # first local row of each slot's mixed block

    for d in range(NCORES):
        lo = SLAB * d
        uc = ucols[(ucols >= lo) & (ucols < lo + SLAB)]
        cols_s = np.full(CAP, -1, np.int64)
        for b in range(TPC):
            blk = uc[(uc - lo) // P == b]
            if len(blk) > BCAP:
                overflow.extend(blk[BCAP:].tolist())
                blk = blk[:BCAP]
            cols_s[BCAP * b : BCAP * b + len(blk)] = blk
        slot_cols[d] = cols_s
        valid = cols_s >= 0

        G = (node_adj[:, np.where(valid, cols_s, 0)] != 0).astype(np.float32)
        G[:, ~valid] = 0.0
        # rotate rows: local row r = absolute row (r + 1024d) mod N
        G = np.concatenate([G[lo:], G[:lo]], axis=0)
        lc = np.where(valid, cols_s - lo, -1)  # local split row (diag) per slot
        G[lc[valid], s_idx[valid]] = 0.0       # zero the diagonal
        block = G[base[None, :] + rows128[:, None], s_idx[None, :]]  # [128, CAP]
        lrow = base[None, :] + rows128[:, None]
        diagL = np.where(lrow < lc[None, :], block, 0.0)
        diagU = np.where(lrow > lc[None, :], block, 0.0)
        G[base[None, :] + rows128[:, None], s_idx[None, :]] = diagL
        # flat [128, 64*512]: adjf[p, 512j + s] = G[128j + p, s]
        adjf = np.ascontiguousarray(
            G.reshape(NT, P, CAP).transpose(1, 0, 2).reshape(P, NT * CAP).astype(FP8)
        )
        in_maps.append(
            {
                "adj": adjf,
                "wmat": _build_wmat(wside, d),
                "diagu": np.ascontiguousarray(diagU.astype(FP8)),
            }
        )

    mult_of = np.zeros(N, np.int64)
    mult_of[ucols] = mult
    over_loss = _host_cols_loss(outputs, targets, node_adj, overflow, mult_of)
    ctx = {"slot_cols": slot_cols, "mult_of": mult_of, "over_loss": over_loss}
    return in_maps, ctx


def _host_cols_loss(outputs, targets, node_adj, cols, mult_of):
    """Reference-exact loss contribution of a few columns (bucket overflow only)."""
    if not cols:
        return 0.0
    cols = np.asarray(cols, np.int64)
    out = np.asarray(outputs, np.float64).reshape(-1)
    pos = np.asarray(targets).reshape(-1) != 0
    A = node_adj[:, cols] != 0
    r = np.arange(N)[:, None]
    A = A & (r != cols[None, :])
    total = 0.0
    for mask in (A & (r < cols[None, :]), A & (r > cols[None, :])):
        cnt = mask.sum(axis=0)
        poscnt = (mask & pos[:, None]).sum(axis=0)
        sumexp = (mask * np.exp(out)[:, None]).sum(axis=0)
        poslogit = (mask * (pos * out)[:, None]).sum(axis=0)
        valid = (cnt > 0) & (poscnt == 1)
        contrib = np.where(
            valid,
            (np.log(np.maximum(sumexp, 1e-300)) - poslogit) / np.maximum(cnt, 1),
            0.0,
        )
        total += (contrib * mult_of[cols]).sum()
    return total


def _combine(stats_list, ctx):
    """Per-core stats [16, CAP] f32 -> scalar loss (f64 math)."""

    def side_contrib(x):
        cnt, poscnt = x[0], x[1]
        poslogit = x[2] + x[3] + x[4]
        sumexp = x[5] + x[6] + x[7]
        valid = (cnt > 0.5) & (np.abs(poscnt - 1.0) < 0.25)
        lse = np.log(np.where(valid, np.maximum(sumexp, 1e-300), 1.0))
        return np.where(valid, (lse - poslogit) / np.maximum(cnt, 1.0), 0.0)

    total = ctx["over_loss"]
    for d, s in enumerate(stats_list):
        x = np.asarray(s, np.float64)
        contrib = side_contrib(x[0:NW]) + side_contrib(x[NW:M])
        cols = ctx["slot_cols"][d]
        valid = cols >= 0
        total += (contrib[valid] * ctx["mult_of"][cols[valid]]).sum()
    return np.array(total, dtype=np.float32)


def _ensure_axon_hooks_stub():
    """bass_utils imports antenv.axon_hooks when tracing is requested via
    env; the module is absent on some images. Provide a no-op stub so the
    import never crashes (hook=None -> bass_utils skips tracing)."""
    import sys
    import types

    try:
        import antenv.axon_hooks  # noqa: F401
    except ImportError:
        mod = types.ModuleType("antenv.axon_hooks")
        state = {"hook": None}
        mod.set_axon_ntff_profile_hook = lambda h: state.__setitem__("hook", h)
        mod.get_axon_ntff_profile_hook = lambda: state["hook"]
        sys.modules["antenv.axon_hooks"] = mod


def _device_stats(in_maps):
    _ensure_axon_hooks_stub()
    from concourse.bass_utils import run_bass_kernel_spmd

    if "nc" not in _BASS_CACHE:
        _BASS_CACHE["nc"] = _build_bass()
    last_exc = None
    for attempt in range(4):
        try:
            res = run_bass_kernel_spmd(
                _BASS_CACHE["nc"], in_maps, core_ids=list(range(NCORES))
            )
            return [r["stats"] for r in res.results]
        except Exception as e:  # transient NRT/accelerator hiccups
            last_exc = e
            try:
                # a fresh PJRT client usually recovers a transiently
                # "unrecoverable" accelerator; mirrors a process restart
                import jax
                import jax.extend.backend as _jeb

                jax.clear_caches()
                _jeb.clear_backends()
            except Exception:
                pass
            import time

            time.sleep(2.0 * (attempt + 1))
    raise last_exc


def _sim_stats(in_maps):
    """Numpy emulation of the device kernel (same inputs), for logic validation."""
    outs = []
    for m in in_maps:
        adjf = m["adj"].astype(np.float32)
        diagu = m["diagu"].astype(np.float32)
        w = m["wmat"].reshape(P, NT + TPC, VW).astype(np.float32)
        acc = np.zeros((M, CAP), np.float32)
        for j in range(NT):
            tile = adjf[:, j * CAP : (j + 1) * CAP]
            if j < TPC:
                c0 = BCAP * j
                acc[:, c0:] += w[:, j, :M].T @ tile[:, c0:]
                acc[:, :c0] += w[:, NT + j, :M].T @ tile[:, :c0]
                acc[:, c0 : c0 + BCAP] += w[:, NT + j, :M].T @ diagu[:, c0 : c0 + BCAP]
            else:
                acc += w[:, j, :M].T @ tile
        outs.append(acc)
    return outs


def kernel(outputs, targets, node_adj, idx_node, _simulate=False):
    in_maps, ctx = _prepare(outputs, targets, node_adj, idx_node)
    stats = _sim_stats(in_maps) if _simulate else _device_stats(in_maps)
    return _combine(stats, ctx)


# revision 18
# speedup vs baseline: 3.9171x; 1.0066x over previous
"""Trainium2 Bass kernel for nn_CELoss_4896262717859 (v3: fp8 DoubleRow).

For each query column c = idx_node[k] of a sparse adjacency matrix (diagonal
zeroed), a cross-entropy-style loss over the "lower" (r < c) and "upper"
(r > c) neighbor sets:

    contrib_side(c) = [cnt>0 and poscnt==1] * (log(sum_r m exp(out_r)) - poslogit) / cnt

All per-column quantities are sums  sum_r adj[r,c] * w[r]  for
w in {1, pos, pos*out, exp(out)} -> tensor-engine matvecs with a triangular
(L/U) split, computed ONLY for the distinct idx_node columns (~3218 of 8192),
then combined with multiplicities on the host (O(N+K)).

Sharding: core d handles the distinct query columns falling in column slab
[1024d, 1024(d+1)).  Within a slab, columns are bucketed by the 128-row block
containing their diagonal (the "mixed" block); each of the 8 buckets is padded
to a fixed 64 slots -> exactly 512 column slots per core, so ONE compiled
program (fixed matmul ranges) serves every core and any input.  Rows are
rotated by 1024d so the mixed blocks always land in local row-tiles 0..7.

Everything streams as fp8e4 (adjacency 0/1 exact; weights hi/mid/lo split
-> ~12 mantissa bits): 4.2 MB/core, and the 56 non-mixed row tiles run as 28
DoubleRow matmul pairs (2 fp8 MACs/cell/cycle).  The mixed 128-row block of
each column is pre-masked on the host: its lower part (rows < c) replaces the
block in the main slab (covered by the L matmul), its upper part goes to a
small separate diagu[128,512] operand (one extra 64-wide matmul per diagonal
tile).  No on-device casts or mask multiplies remain.

Any bucket overflow beyond 64 distinct columns (never happens for uniform
idx_node; p<1% per bucket) falls back to a tiny host-side computation for the
overflowed columns only.
"""

import numpy as np
import ml_dtypes

N = 8192
K = 4096
NCORES = 8
SLAB = N // NCORES        # 1024 columns per slab
P = 128                   # partition / tile edge
NT = N // P               # 64 row tiles
TPC = SLAB // P           # 8 diagonal (mixed) tiles per core
NW = 8                    # weights per side: {1, pos, pl_h, pl_m, pl_l, e_h, e_m, e_l}
M = 2 * NW                # 16 psum partitions (L half = 0:8, U half = 8:16)
VW = 16                   # weight-variant stride (cols); == M, and 16B for fp8
CAP = 448                 # column slots per core (one psum bank)
BCAP = CAP // TPC         # 56 slots per 128-row bucket
CHUNK_TILES = (2, 2, 4, 8, 8, 12, 12, 16)   # row tiles per DMA chunk
CHUNK_ENGS = ("sync",) * 8                  # issuing ring per chunk

BF16 = ml_dtypes.bfloat16
FP8 = ml_dtypes.float8_e4m3fn

_BASS_CACHE = {}


def _build_bass():
    import concourse.tile as tile
    import concourse.mybir as mybir
    from concourse import bacc

    f8 = mybir.dt.float8e4
    DR = mybir.MatmulPerfMode.DoubleRow
    # Bacc (not raw Bass): its compile() runs generate_event_semaphores,
    # which splits multi-sem waits — TRN2 instructions hold at most one.
    nc = bacc.Bacc("TRN2")
    adj = nc.dram_tensor("adj", [P, NT * CAP], f8, kind="ExternalInput")
    wmat = nc.dram_tensor("wmat", [P, (NT + TPC) * VW], f8, kind="ExternalInput")
    diagu = nc.dram_tensor("diagu", [P, CAP], f8, kind="ExternalInput")
    stats = nc.dram_tensor("stats", [M, CAP], mybir.dt.float32, kind="ExternalOutput")

    with tile.TileContext(nc) as tc:
        with (
            tc.tile_pool(name="singles", bufs=1) as singles,
            tc.tile_pool(name="psum", bufs=1, space="PSUM") as psum_pool,
        ):
            # weights first: every matmul needs them.  DMAs round-robin over
            # THREE descriptor rings (sync HWDGE / scalar HWDGE / gpsimd
            # SWDGE) so descriptor generation and ring draining overlap; the
            # leading chunks are small so the first matmuls start early.
            asb = singles.tile([P, NT + TPC, VW], f8)
            nc.sync.dma_start(out=asb, in_=wmat[:, :])
            dsb = singles.tile([P, CAP], f8)
            nc.scalar.dma_start(out=dsb, in_=diagu[:, :])
            engs = {"sync": nc.sync, "scalar": nc.scalar, "gpsimd": nc.gpsimd}
            chunks = []   # (tile, first_tile, ntiles)
            t0 = 0
            for ci, nt in enumerate(CHUNK_TILES):
                t = singles.tile([P, nt, CAP], f8, name=f"ch{ci}")
                engs[CHUNK_ENGS[ci]].dma_start(
                    out=t, in_=adj[:, t0 * CAP : (t0 + nt) * CAP]
                )
                chunks.append((t, t0, nt))
                t0 += nt
            assert t0 == NT

            acc = psum_pool.tile([M, CAP], mybir.dt.float32, name="acc")

            def wv(v, n=1):
                return asb[:, v : v + n, :]

            def du(b):  # diagu strip for mixed block b
                return dsb[:, BCAP * b : BCAP * (b + 1)]

            def chunk_rhs(j, n=1):
                for t, t0, nt in chunks:
                    if t0 <= j and j + n <= t0 + nt:
                        return t[:, j - t0 : j - t0 + n, :]
                raise AssertionError(f"tile {j}+{n} spans chunks")

            def mm(out_ap, w, rhs, **kw):
                nc.tensor.matmul(out_ap, w, rhs, skip_group_check=True,
                                 start=kw.pop("start", False),
                                 stop=kw.pop("stop", False), **kw)

            # Diagonal (mixed) tiles, DoubleRow-paired on their shared spans.
            # Tile j's L span is [64j, 512) (its own mixed bucket holds
            # host-pre-masked lower data), U span is [0, 64j); the mixed
            # bucket's upper part comes from the diagu strips.
            for j in range(0, TPC, 2):
                a, b = BCAP * j, BCAP * (j + 1)
                rhs2 = chunk_rhs(j, 2)
                # shared L span of the pair
                mm(acc[:, b:CAP], wv(j, 2), rhs2[:, :, b:CAP],
                   start=(j == 0), perf_mode=DR)
                # tile j's extra L strip (its own bucket)
                mm(acc[:, a:b], wv(j), chunk_rhs(j)[:, :, a:b])
                if j > 0:
                    # shared U span of the pair
                    mm(acc[:, 0:a], wv(NT + j, 2), rhs2[:, :, 0:a], perf_mode=DR)
                # tile j+1's extra U strip (tile j's bucket columns)
                mm(acc[:, a:b], wv(NT + j + 1), chunk_rhs(j + 1)[:, :, a:b])
                # upper parts of the mixed buckets themselves
                mm(acc[:, a:b], wv(NT + j), du(j))
                mm(acc[:, b : b + BCAP], wv(NT + j + 1), du(j + 1))
            for j in range(TPC, NT, 2):
                mm(acc[:, :], wv(j, 2), chunk_rhs(j, 2),
                   stop=(j == NT - 2), perf_mode=DR)

            out_sb = singles.tile([M, CAP], mybir.dt.float32)
            half = CAP // 2
            nc.vector.tensor_copy(out_sb[:, half:], acc[:, half:])
            nc.scalar.copy(out_sb[:, 0:half], acc[:, 0:half])
            nc.sync.dma_start(out=stats[:, 0:half], in_=out_sb[:, 0:half])
            nc.scalar.dma_start(out=stats[:, half:], in_=out_sb[:, half:])

    nc.compile()
    return nc


def _split_fp8(v, terms=3):
    """Split f64 vector into `terms` fp8 values summing to ~v (12 mantissa bits)."""
    out = []
    r = np.asarray(v, np.float64)
    for _ in range(terms):
        t = r.astype(FP8)
        out.append(t)
        r = r - t.astype(np.float64)
    return out


def _make_wside(outputs, targets):
    """Per-row weight table [N, 8] fp8."""
    out = np.asarray(outputs, np.float64).reshape(-1)
    pos = (np.asarray(targets).reshape(-1) != 0).astype(np.float64)
    cols = [np.ones(N, FP8), pos.astype(FP8)]
    cols += _split_fp8(pos * out)
    cols += _split_fp8(np.exp(out))
    return np.stack(cols, axis=1).astype(FP8)  # [N, 8]


def _build_wmat(wside, core):
    """Per-core weight variants [128, (64+8)*16] fp8.

    Variant j (j<64): weights for local row tile j (absolute tile (8*core+j)%64).
      j < 8  -> L-only variant (diag tiles; U-only twin stored at 64+j)
      j >= 8 -> single variant, L or U half per the tile's position vs the slab
    """
    w = np.zeros((P, NT + TPC, VW), dtype=FP8)
    for j in range(NT):
        t = (TPC * core + j) % NT
        rows = wside[t * P : (t + 1) * P, :]
        if j < TPC:
            w[:, j, 0:NW] = rows
            w[:, NT + j, NW:M] = rows
        elif j < NT - TPC * core:
            w[:, j, NW:M] = rows  # rows above slab columns -> U
        else:
            w[:, j, 0:NW] = rows  # wrapped rows below slab columns -> L
    return np.ascontiguousarray(w.reshape(P, (NT + TPC) * VW))


def _prepare(outputs, targets, node_adj, idx_node):
    """Build per-core in_maps + combine context (slot->column map, multiplicities,
    host-computed contribution of any bucket-overflow columns)."""
    node_adj = np.asarray(node_adj)
    idx = np.asarray(idx_node).reshape(-1).astype(np.int64)
    ucols, mult = np.unique(idx, return_counts=True)
    wside = _make_wside(outputs, targets)

    in_maps = []
    slot_cols = np.full((NCORES, CAP), -1, np.int64)
    overflow = []
    rows128 = np.arange(P)
    s_idx = np.arange(CAP)
    base = P * (s_idx // BCAP)  # first local row of each slot's mixed block

    for d in range(NCORES):
        lo = SLAB * d
        uc = ucols[(ucols >= lo) & (ucols < lo + SLAB)]
        cols_s = np.full(CAP, -1, np.int64)
        for b in range(TPC):
            blk = uc[(uc - lo) // P == b]
            if len(blk) > BCAP:
                overflow.extend(blk[BCAP:].tolist())
                blk = blk[:BCAP]
            cols_s[BCAP * b : BCAP * b + len(blk)] = blk
        slot_cols[d] = cols_s
        valid = cols_s >= 0

        G = (node_adj[:, np.where(valid, cols_s, 0)] != 0).astype(np.float32)
        G[:, ~valid] = 0.0
        # rotate rows: local row r = absolute row (r + 1024d) mod N
        G = np.concatenate([G[lo:], G[:lo]], axis=0)
        lc = np.where(valid, cols_s - lo, -1)  # local split row (diag) per slot
        G[lc[valid], s_idx[valid]] = 0.0       # zero the diagonal
        block = G[base[None, :] + rows128[:, None], s_idx[None, :]]  # [128, CAP]
        lrow = base[None, :] + rows128[:, None]
        diagL = np.where(lrow < lc[None, :], block, 0.0)
        diagU = np.where(lrow > lc[None, :], block, 0.0)
        G[base[None, :] + rows128[:, None], s_idx[None, :]] = diagL
        # flat [128, 64*512]: adjf[p, 512j + s] = G[128j + p, s]
        adjf = np.ascontiguousarray(
            G.reshape(NT, P, CAP).transpose(1, 0, 2).reshape(P, NT * CAP).astype(FP8)
        )
        in_maps.append(
            {
                "adj": adjf,
                "wmat": _build_wmat(wside, d),
                "diagu": np.ascontiguousarray(diagU.astype(FP8)),
            }
        )

    mult_of = np.zeros(N, np.int64)
    mult_of[ucols] = mult
    over_loss = _host_cols_loss(outputs, targets, node_adj, overflow, mult_of)
    ctx = {"slot_cols": slot_cols, "mult_of": mult_of, "over_loss": over_loss}
    return in_maps, ctx


def _host_cols_loss(outputs, targets, node_adj, cols, mult_of):
    """Reference-exact loss contribution of a few columns (bucket overflow only)."""
    if not cols:
        return 0.0
    cols = np.asarray(cols, np.int64)
    out = np.asarray(outputs, np.float64).reshape(-1)
    pos = np.asarray(targets).reshape(-1) != 0
    A = node_adj[:, cols] != 0
    r = np.arange(N)[:, None]
    A = A & (r != cols[None, :])
    total = 0.0
    for mask in (A & (r < cols[None, :]), A & (r > cols[None, :])):
        cnt = mask.sum(axis=0)
        poscnt = (mask & pos[:, None]).sum(axis=0)
        sumexp = (mask * np.exp(out)[:, None]).sum(axis=0)
        poslogit = (mask * (pos * out)[:, None]).sum(axis=0)
        valid = (cnt > 0) & (poscnt == 1)
        contrib = np.where(
            valid,
            (np.log(np.maximum(sumexp, 1e-300)) - poslogit) / np.maximum(cnt, 1),
            0.0,
        )
        total += (contrib * mult_of[cols]).sum()
    return total


def _combine(stats_list, ctx):
    """Per-core stats [16, CAP] f32 -> scalar loss (f64 math)."""

    def side_contrib(x):
        cnt, poscnt = x[0], x[1]
        poslogit = x[2] + x[3] + x[4]
        sumexp = x[5] + x[6] + x[7]
        valid = (cnt > 0.5) & (np.abs(poscnt - 1.0) < 0.25)
        lse = np.log(np.where(valid, np.maximum(sumexp, 1e-300), 1.0))
        return np.where(valid, (lse - poslogit) / np.maximum(cnt, 1.0), 0.0)

    total = ctx["over_loss"]
    for d, s in enumerate(stats_list):
        x = np.asarray(s, np.float64)
        contrib = side_contrib(x[0:NW]) + side_contrib(x[NW:M])
        cols = ctx["slot_cols"][d]
        valid = cols >= 0
        total += (contrib[valid] * ctx["mult_of"][cols[valid]]).sum()
    return np.array(total, dtype=np.float32)


def _ensure_axon_hooks_stub():
    """bass_utils imports antenv.axon_hooks when tracing is requested via
    env; the module is absent on some images. Provide a no-op stub so the
    import never crashes (hook=None -> bass_utils skips tracing)."""
    import sys
    import types

    try:
        import antenv.axon_hooks  # noqa: F401
    except ImportError:
        mod = types.ModuleType("antenv.axon_hooks")
        state = {"hook": None}
        mod.set_axon_ntff_profile_hook = lambda h: state.__setitem__("hook", h)
        mod.get_axon_ntff_profile_hook = lambda: state["hook"]
        sys.modules["antenv.axon_hooks"] = mod


def _device_stats(in_maps):
    _ensure_axon_hooks_stub()
    from concourse.bass_utils import run_bass_kernel_spmd

    if "nc" not in _BASS_CACHE:
        _BASS_CACHE["nc"] = _build_bass()
    last_exc = None
    for attempt in range(4):
        try:
            res = run_bass_kernel_spmd(
                _BASS_CACHE["nc"], in_maps, core_ids=list(range(NCORES))
            )
            return [r["stats"] for r in res.results]
        except Exception as e:  # transient NRT/accelerator hiccups
            last_exc = e
            try:
                # a fresh PJRT client usually recovers a transiently
                # "unrecoverable" accelerator; mirrors a process restart
                import jax
                import jax.extend.backend as _jeb

                jax.clear_caches()
                _jeb.clear_backends()
            except Exception:
                pass
            import time

            time.sleep(2.0 * (attempt + 1))
    raise last_exc


def _sim_stats(in_maps):
    """Numpy emulation of the device kernel (same inputs), for logic validation."""
    outs = []
    for m in in_maps:
        adjf = m["adj"].astype(np.float32)
        diagu = m["diagu"].astype(np.float32)
        w = m["wmat"].reshape(P, NT + TPC, VW).astype(np.float32)
        acc = np.zeros((M, CAP), np.float32)
        for j in range(NT):
            tile = adjf[:, j * CAP : (j + 1) * CAP]
            if j < TPC:
                c0 = BCAP * j
                acc[:, c0:] += w[:, j, :M].T @ tile[:, c0:]
                acc[:, :c0] += w[:, NT + j, :M].T @ tile[:, :c0]
                acc[:, c0 : c0 + BCAP] += w[:, NT + j, :M].T @ diagu[:, c0 : c0 + BCAP]
            else:
                acc += w[:, j, :M].T @ tile
        outs.append(acc)
    return outs


def kernel(outputs, targets, node_adj, idx_node, _simulate=False):
    in_maps, ctx = _prepare(outputs, targets, node_adj, idx_node)
    stats = _sim_stats(in_maps) if _simulate else _device_stats(in_maps)
    return _combine(stats, ctx)


# revision 23
# speedup vs baseline: 4.0071x; 1.0230x over previous
"""Trainium2 Bass kernel for nn_CELoss_4896262717859 (v3: fp8 DoubleRow).

For each query column c = idx_node[k] of a sparse adjacency matrix (diagonal
zeroed), a cross-entropy-style loss over the "lower" (r < c) and "upper"
(r > c) neighbor sets:

    contrib_side(c) = [cnt>0 and poscnt==1] * (log(sum_r m exp(out_r)) - poslogit) / cnt

All per-column quantities are sums  sum_r adj[r,c] * w[r]  for
w in {1, pos, pos*out, exp(out)} -> tensor-engine matvecs with a triangular
(L/U) split, computed ONLY for the distinct idx_node columns (~3218 of 8192),
then combined with multiplicities on the host (O(N+K)).

Sharding: core d handles the distinct query columns falling in column slab
[1024d, 1024(d+1)).  Within a slab, columns are bucketed by the 128-row block
containing their diagonal (the "mixed" block); each of the 8 buckets is padded
to a fixed 64 slots -> exactly 512 column slots per core, so ONE compiled
program (fixed matmul ranges) serves every core and any input.  Rows are
rotated by 1024d so the mixed blocks always land in local row-tiles 0..7.

Everything streams as fp8e4 (adjacency 0/1 exact; weights hi/mid/lo split
-> ~12 mantissa bits): 4.2 MB/core, and the 56 non-mixed row tiles run as 28
DoubleRow matmul pairs (2 fp8 MACs/cell/cycle).  The mixed 128-row block of
each column is pre-masked on the host: its lower part (rows < c) replaces the
block in the main slab (covered by the L matmul), its upper part goes to a
small separate diagu[128,512] operand (one extra 64-wide matmul per diagonal
tile).  No on-device casts or mask multiplies remain.

Any bucket overflow beyond 64 distinct columns (never happens for uniform
idx_node; p<1% per bucket) falls back to a tiny host-side computation for the
overflowed columns only.
"""

import numpy as np
import ml_dtypes

N = 8192
K = 4096
NCORES = 8
SLAB = N // NCORES        # 1024 columns per slab
P = 128                   # partition / tile edge
NT = N // P               # 64 row tiles
TPC = SLAB // P           # 8 diagonal (mixed) tiles per core
NW = 8                    # weights per side: {1, pos, pl_h, pl_m, pl_l, e_h, e_m, e_l}
M = 2 * NW                # 16 psum partitions (L half = 0:8, U half = 8:16)
VW = 16                   # weight-variant stride (cols); == M, and 16B for fp8
CAP = 448                 # column slots per core (one psum bank)
BCAP = CAP // TPC         # 56 slots per 128-row bucket
CHUNK_TILES = (2, 2, 4, 8, 12, 16, 16, 4)   # row tiles per DMA chunk
CHUNK_ENGS = ("sync",) * 8                  # issuing ring per chunk

BF16 = ml_dtypes.bfloat16
FP8 = ml_dtypes.float8_e4m3fn

_BASS_CACHE = {}


def _build_bass():
    import concourse.tile as tile
    import concourse.mybir as mybir
    from concourse import bacc

    f8 = mybir.dt.float8e4
    DR = mybir.MatmulPerfMode.DoubleRow
    # Bacc (not raw Bass): its compile() runs generate_event_semaphores,
    # which splits multi-sem waits — TRN2 instructions hold at most one.
    nc = bacc.Bacc("TRN2")
    adjs = [
        nc.dram_tensor(f"adj{ci}", [P, nt * CAP], f8, kind="ExternalInput")
        for ci, nt in enumerate(CHUNK_TILES)
    ]
    wmat = nc.dram_tensor("wmat", [P, (NT + TPC) * VW], f8, kind="ExternalInput")
    diagu = nc.dram_tensor("diagu", [P, CAP], f8, kind="ExternalInput")
    stats = nc.dram_tensor("stats", [M, CAP], mybir.dt.float32, kind="ExternalOutput")

    with tile.TileContext(nc) as tc:
        with (
            tc.tile_pool(name="singles", bufs=1) as singles,
            tc.tile_pool(name="psum", bufs=1, space="PSUM") as psum_pool,
        ):
            # weights first: every matmul needs them.  DMAs round-robin over
            # THREE descriptor rings (sync HWDGE / scalar HWDGE / gpsimd
            # SWDGE) so descriptor generation and ring draining overlap; the
            # leading chunks are small so the first matmuls start early.
            asb = singles.tile([P, NT + TPC, VW], f8)
            nc.sync.dma_start(out=asb, in_=wmat[:, :])
            dsb = singles.tile([P, CAP], f8)
            nc.scalar.dma_start(out=dsb, in_=diagu[:, :])
            engs = {"sync": nc.sync, "scalar": nc.scalar, "gpsimd": nc.gpsimd}
            chunks = []   # (tile, first_tile, ntiles)
            t0 = 0
            for ci, nt in enumerate(CHUNK_TILES):
                t = singles.tile([P, nt, CAP], f8, name=f"ch{ci}")
                engs[CHUNK_ENGS[ci]].dma_start(out=t, in_=adjs[ci][:, :])
                chunks.append((t, t0, nt))
                t0 += nt
            assert t0 == NT

            acc = psum_pool.tile([M, CAP], mybir.dt.float32, name="acc")

            def wv(v, n=1):
                return asb[:, v : v + n, :]

            def du(b):  # diagu strip for mixed block b
                return dsb[:, BCAP * b : BCAP * (b + 1)]

            def chunk_rhs(j, n=1):
                for t, t0, nt in chunks:
                    if t0 <= j and j + n <= t0 + nt:
                        return t[:, j - t0 : j - t0 + n, :]
                raise AssertionError(f"tile {j}+{n} spans chunks")

            def mm(out_ap, w, rhs, **kw):
                nc.tensor.matmul(out_ap, w, rhs, skip_group_check=True,
                                 start=kw.pop("start", False),
                                 stop=kw.pop("stop", False), **kw)

            # Diagonal (mixed) tiles, DoubleRow-paired on their shared spans.
            # Tile j's L span is [64j, 512) (its own mixed bucket holds
            # host-pre-masked lower data), U span is [0, 64j); the mixed
            # bucket's upper part comes from the diagu strips.
            for j in range(0, TPC, 2):
                a, b = BCAP * j, BCAP * (j + 1)
                rhs2 = chunk_rhs(j, 2)
                # shared L span of the pair
                mm(acc[:, b:CAP], wv(j, 2), rhs2[:, :, b:CAP],
                   start=(j == 0), perf_mode=DR)
                # tile j's extra L strip (its own bucket)
                mm(acc[:, a:b], wv(j), chunk_rhs(j)[:, :, a:b])
                if j > 0:
                    # shared U span of the pair
                    mm(acc[:, 0:a], wv(NT + j, 2), rhs2[:, :, 0:a], perf_mode=DR)
                # tile j+1's extra U strip (tile j's bucket columns)
                mm(acc[:, a:b], wv(NT + j + 1), chunk_rhs(j + 1)[:, :, a:b])
                # upper parts of the mixed buckets themselves
                mm(acc[:, a:b], wv(NT + j), du(j))
                mm(acc[:, b : b + BCAP], wv(NT + j + 1), du(j + 1))
            for j in range(TPC, NT, 2):
                mm(acc[:, :], wv(j, 2), chunk_rhs(j, 2),
                   stop=(j == NT - 2), perf_mode=DR)

            out_sb = singles.tile([M, CAP], mybir.dt.float32)
            half = CAP // 2
            nc.vector.tensor_copy(out_sb[:, half:], acc[:, half:])
            nc.scalar.copy(out_sb[:, 0:half], acc[:, 0:half])
            nc.sync.dma_start(out=stats[:, 0:half], in_=out_sb[:, 0:half])
            nc.scalar.dma_start(out=stats[:, half:], in_=out_sb[:, half:])

    nc.compile()
    return nc


def _split_fp8(v, terms=3):
    """Split f64 vector into `terms` fp8 values summing to ~v (12 mantissa bits)."""
    out = []
    r = np.asarray(v, np.float64)
    for _ in range(terms):
        t = r.astype(FP8)
        out.append(t)
        r = r - t.astype(np.float64)
    return out


def _make_wside(outputs, targets):
    """Per-row weight table [N, 8] fp8."""
    out = np.asarray(outputs, np.float64).reshape(-1)
    pos = (np.asarray(targets).reshape(-1) != 0).astype(np.float64)
    cols = [np.ones(N, FP8), pos.astype(FP8)]
    cols += _split_fp8(pos * out)
    cols += _split_fp8(np.exp(out))
    return np.stack(cols, axis=1).astype(FP8)  # [N, 8]


def _build_wmat(wside, core):
    """Per-core weight variants [128, (64+8)*16] fp8.

    Variant j (j<64): weights for local row tile j (absolute tile (8*core+j)%64).
      j < 8  -> L-only variant (diag tiles; U-only twin stored at 64+j)
      j >= 8 -> single variant, L or U half per the tile's position vs the slab
    """
    w = np.zeros((P, NT + TPC, VW), dtype=FP8)
    for j in range(NT):
        t = (TPC * core + j) % NT
        rows = wside[t * P : (t + 1) * P, :]
        if j < TPC:
            w[:, j, 0:NW] = rows
            w[:, NT + j, NW:M] = rows
        elif j < NT - TPC * core:
            w[:, j, NW:M] = rows  # rows above slab columns -> U
        else:
            w[:, j, 0:NW] = rows  # wrapped rows below slab columns -> L
    return np.ascontiguousarray(w.reshape(P, (NT + TPC) * VW))


def _prepare(outputs, targets, node_adj, idx_node):
    """Build per-core in_maps + combine context (slot->column map, multiplicities,
    host-computed contribution of any bucket-overflow columns)."""
    node_adj = np.asarray(node_adj)
    idx = np.asarray(idx_node).reshape(-1).astype(np.int64)
    ucols, mult = np.unique(idx, return_counts=True)
    wside = _make_wside(outputs, targets)

    in_maps = []
    slot_cols = np.full((NCORES, CAP), -1, np.int64)
    overflow = []
    rows128 = np.arange(P)
    s_idx = np.arange(CAP)
    base = P * (s_idx // BCAP)  # first local row of each slot's mixed block

    for d in range(NCORES):
        lo = SLAB * d
        uc = ucols[(ucols >= lo) & (ucols < lo + SLAB)]
        cols_s = np.full(CAP, -1, np.int64)
        for b in range(TPC):
            blk = uc[(uc - lo) // P == b]
            if len(blk) > BCAP:
                overflow.extend(blk[BCAP:].tolist())
                blk = blk[:BCAP]
            cols_s[BCAP * b : BCAP * b + len(blk)] = blk
        slot_cols[d] = cols_s
        valid = cols_s >= 0

        G = (node_adj[:, np.where(valid, cols_s, 0)] != 0).astype(np.float32)
        G[:, ~valid] = 0.0
        # rotate rows: local row r = absolute row (r + 1024d) mod N
        G = np.concatenate([G[lo:], G[:lo]], axis=0)
        lc = np.where(valid, cols_s - lo, -1)  # local split row (diag) per slot
        G[lc[valid], s_idx[valid]] = 0.0       # zero the diagonal
        block = G[base[None, :] + rows128[:, None], s_idx[None, :]]  # [128, CAP]
        lrow = base[None, :] + rows128[:, None]
        diagL = np.where(lrow < lc[None, :], block, 0.0)
        diagU = np.where(lrow > lc[None, :], block, 0.0)
        G[base[None, :] + rows128[:, None], s_idx[None, :]] = diagL
        # tile-major flat layout: adjf[p, CAP*j + s] = G[128j + p, s], split
        # into one contiguous dram tensor per DMA chunk
        adjf = G.reshape(NT, P, CAP).transpose(1, 0, 2).reshape(P, NT * CAP).astype(FP8)
        im = {
            "wmat": _build_wmat(wside, d),
            "diagu": np.ascontiguousarray(diagU.astype(FP8)),
        }
        t0 = 0
        for ci, nt in enumerate(CHUNK_TILES):
            im[f"adj{ci}"] = np.ascontiguousarray(
                adjf[:, t0 * CAP : (t0 + nt) * CAP]
            )
            t0 += nt
        in_maps.append(im)

    mult_of = np.zeros(N, np.int64)
    mult_of[ucols] = mult
    over_loss = _host_cols_loss(outputs, targets, node_adj, overflow, mult_of)
    ctx = {"slot_cols": slot_cols, "mult_of": mult_of, "over_loss": over_loss}
    return in_maps, ctx


def _host_cols_loss(outputs, targets, node_adj, cols, mult_of):
    """Reference-exact loss contribution of a few columns (bucket overflow only)."""
    if not cols:
        return 0.0
    cols = np.asarray(cols, np.int64)
    out = np.asarray(outputs, np.float64).reshape(-1)
    pos = np.asarray(targets).reshape(-1) != 0
    A = node_adj[:, cols] != 0
    r = np.arange(N)[:, None]
    A = A & (r != cols[None, :])
    total = 0.0
    for mask in (A & (r < cols[None, :]), A & (r > cols[None, :])):
        cnt = mask.sum(axis=0)
        poscnt = (mask & pos[:, None]).sum(axis=0)
        sumexp = (mask * np.exp(out)[:, None]).sum(axis=0)
        poslogit = (mask * (pos * out)[:, None]).sum(axis=0)
        valid = (cnt > 0) & (poscnt == 1)
        contrib = np.where(
            valid,
            (np.log(np.maximum(sumexp, 1e-300)) - poslogit) / np.maximum(cnt, 1),
            0.0,
        )
        total += (contrib * mult_of[cols]).sum()
    return total


def _combine(stats_list, ctx):
    """Per-core stats [16, CAP] f32 -> scalar loss (f64 math)."""

    def side_contrib(x):
        cnt, poscnt = x[0], x[1]
        poslogit = x[2] + x[3] + x[4]
        sumexp = x[5] + x[6] + x[7]
        valid = (cnt > 0.5) & (np.abs(poscnt - 1.0) < 0.25)
        lse = np.log(np.where(valid, np.maximum(sumexp, 1e-300), 1.0))
        return np.where(valid, (lse - poslogit) / np.maximum(cnt, 1.0), 0.0)

    total = ctx["over_loss"]
    for d, s in enumerate(stats_list):
        x = np.asarray(s, np.float64)
        contrib = side_contrib(x[0:NW]) + side_contrib(x[NW:M])
        cols = ctx["slot_cols"][d]
        valid = cols >= 0
        total += (contrib[valid] * ctx["mult_of"][cols[valid]]).sum()
    return np.array(total, dtype=np.float32)


def _ensure_axon_hooks_stub():
    """bass_utils imports antenv.axon_hooks when tracing is requested via
    env; the module is absent on some images. Provide a no-op stub so the
    import never crashes (hook=None -> bass_utils skips tracing)."""
    import sys
    import types

    try:
        import antenv.axon_hooks  # noqa: F401
    except ImportError:
        mod = types.ModuleType("antenv.axon_hooks")
        state = {"hook": None}
        mod.set_axon_ntff_profile_hook = lambda h: state.__setitem__("hook", h)
        mod.get_axon_ntff_profile_hook = lambda: state["hook"]
        sys.modules["antenv.axon_hooks"] = mod


def _device_stats(in_maps):
    _ensure_axon_hooks_stub()
    from concourse.bass_utils import run_bass_kernel_spmd

    if "nc" not in _BASS_CACHE:
        _BASS_CACHE["nc"] = _build_bass()
    last_exc = None
    for attempt in range(4):
        try:
            res = run_bass_kernel_spmd(
                _BASS_CACHE["nc"], in_maps, core_ids=list(range(NCORES))
            )
            return [r["stats"] for r in res.results]
        except Exception as e:  # transient NRT/accelerator hiccups
            last_exc = e
            try:
                # a fresh PJRT client usually recovers a transiently
                # "unrecoverable" accelerator; mirrors a process restart
                import jax
                import jax.extend.backend as _jeb

                jax.clear_caches()
                _jeb.clear_backends()
            except Exception:
                pass
            import time

            time.sleep(2.0 * (attempt + 1))
    raise last_exc


def _sim_stats(in_maps):
    """Numpy emulation of the device kernel (same inputs), for logic validation."""
    outs = []
    for m in in_maps:
        adjf = np.concatenate(
            [m[f"adj{ci}"] for ci in range(len(CHUNK_TILES))], axis=1
        ).astype(np.float32)
        diagu = m["diagu"].astype(np.float32)
        w = m["wmat"].reshape(P, NT + TPC, VW).astype(np.float32)
        acc = np.zeros((M, CAP), np.float32)
        for j in range(NT):
            tile = adjf[:, j * CAP : (j + 1) * CAP]
            if j < TPC:
                c0 = BCAP * j
                acc[:, c0:] += w[:, j, :M].T @ tile[:, c0:]
                acc[:, :c0] += w[:, NT + j, :M].T @ tile[:, :c0]
                acc[:, c0 : c0 + BCAP] += w[:, NT + j, :M].T @ diagu[:, c0 : c0 + BCAP]
            else:
                acc += w[:, j, :M].T @ tile
        outs.append(acc)
    return outs


def kernel(outputs, targets, node_adj, idx_node, _simulate=False):
    in_maps, ctx = _prepare(outputs, targets, node_adj, idx_node)
    stats = _sim_stats(in_maps) if _simulate else _device_stats(in_maps)
    return _combine(stats, ctx)


# revision 24
# speedup vs baseline: 4.1975x; 1.0475x over previous
"""Trainium2 Bass kernel for nn_CELoss_4896262717859 (v3: fp8 DoubleRow).

For each query column c = idx_node[k] of a sparse adjacency matrix (diagonal
zeroed), a cross-entropy-style loss over the "lower" (r < c) and "upper"
(r > c) neighbor sets:

    contrib_side(c) = [cnt>0 and poscnt==1] * (log(sum_r m exp(out_r)) - poslogit) / cnt

All per-column quantities are sums  sum_r adj[r,c] * w[r]  for
w in {1, pos, pos*out, exp(out)} -> tensor-engine matvecs with a triangular
(L/U) split, computed ONLY for the distinct idx_node columns (~3218 of 8192),
then combined with multiplicities on the host (O(N+K)).

Sharding: core d handles the distinct query columns falling in column slab
[1024d, 1024(d+1)).  Within a slab, columns are bucketed by the 128-row block
containing their diagonal (the "mixed" block); each of the 8 buckets is padded
to a fixed 64 slots -> exactly 512 column slots per core, so ONE compiled
program (fixed matmul ranges) serves every core and any input.  Rows are
rotated by 1024d so the mixed blocks always land in local row-tiles 0..7.

Everything streams as fp8e4 (adjacency 0/1 exact; weights hi/mid/lo split
-> ~12 mantissa bits): 4.2 MB/core, and the 56 non-mixed row tiles run as 28
DoubleRow matmul pairs (2 fp8 MACs/cell/cycle).  The mixed 128-row block of
each column is pre-masked on the host: its lower part (rows < c) replaces the
block in the main slab (covered by the L matmul), its upper part goes to a
small separate diagu[128,512] operand (one extra 64-wide matmul per diagonal
tile).  No on-device casts or mask multiplies remain.

Any bucket overflow beyond 64 distinct columns (never happens for uniform
idx_node; p<1% per bucket) falls back to a tiny host-side computation for the
overflowed columns only.
"""

import numpy as np
import ml_dtypes

N = 8192
K = 4096
NCORES = 8
SLAB = N // NCORES        # 1024 columns per slab
P = 128                   # partition / tile edge
NT = N // P               # 64 row tiles
TPC = SLAB // P           # 8 diagonal (mixed) tiles per core
NW = 8                    # weights per side: {1, pos, pl_h, pl_m, pl_l, e_h, e_m, e_l}
M = 2 * NW                # 16 psum partitions (L half = 0:8, U half = 8:16)
VW = 16                   # weight-variant stride (cols); == M, and 16B for fp8
CAP = 448                 # column slots per core (one psum bank)
BCAP = CAP // TPC         # 56 slots per 128-row bucket
CHUNK_TILES = (2, 2, 4, 8, 12, 16, 16, 4)   # row tiles per DMA chunk
CHUNK_ENGS = ("gpsimd",) * 8                # issuing ring per chunk

BF16 = ml_dtypes.bfloat16
FP8 = ml_dtypes.float8_e4m3fn

_BASS_CACHE = {}


def _build_bass():
    import concourse.tile as tile
    import concourse.mybir as mybir
    from concourse import bacc

    f8 = mybir.dt.float8e4
    DR = mybir.MatmulPerfMode.DoubleRow
    # Bacc (not raw Bass): its compile() runs generate_event_semaphores,
    # which splits multi-sem waits — TRN2 instructions hold at most one.
    nc = bacc.Bacc("TRN2")
    adjs = [
        nc.dram_tensor(f"adj{ci}", [P, nt * CAP], f8, kind="ExternalInput")
        for ci, nt in enumerate(CHUNK_TILES)
    ]
    wmat = nc.dram_tensor("wmat", [P, (NT + TPC) * VW], f8, kind="ExternalInput")
    diagu = nc.dram_tensor("diagu", [P, CAP], f8, kind="ExternalInput")
    stats = nc.dram_tensor("stats", [M, CAP], mybir.dt.float32, kind="ExternalOutput")

    with tile.TileContext(nc) as tc:
        with (
            tc.tile_pool(name="singles", bufs=1) as singles,
            tc.tile_pool(name="psum", bufs=1, space="PSUM") as psum_pool,
        ):
            # weights first: every matmul needs them.  DMAs round-robin over
            # THREE descriptor rings (sync HWDGE / scalar HWDGE / gpsimd
            # SWDGE) so descriptor generation and ring draining overlap; the
            # leading chunks are small so the first matmuls start early.
            asb = singles.tile([P, NT + TPC, VW], f8)
            nc.sync.dma_start(out=asb, in_=wmat[:, :])
            dsb = singles.tile([P, CAP], f8)
            nc.scalar.dma_start(out=dsb, in_=diagu[:, :])
            engs = {"sync": nc.sync, "scalar": nc.scalar, "gpsimd": nc.gpsimd}
            chunks = []   # (tile, first_tile, ntiles)
            t0 = 0
            for ci, nt in enumerate(CHUNK_TILES):
                t = singles.tile([P, nt, CAP], f8, name=f"ch{ci}")
                engs[CHUNK_ENGS[ci]].dma_start(out=t, in_=adjs[ci][:, :])
                chunks.append((t, t0, nt))
                t0 += nt
            assert t0 == NT

            acc = psum_pool.tile([M, CAP], mybir.dt.float32, name="acc")

            def wv(v, n=1):
                return asb[:, v : v + n, :]

            def du(b):  # diagu strip for mixed block b
                return dsb[:, BCAP * b : BCAP * (b + 1)]

            def chunk_rhs(j, n=1):
                for t, t0, nt in chunks:
                    if t0 <= j and j + n <= t0 + nt:
                        return t[:, j - t0 : j - t0 + n, :]
                raise AssertionError(f"tile {j}+{n} spans chunks")

            def mm(out_ap, w, rhs, **kw):
                nc.tensor.matmul(out_ap, w, rhs, skip_group_check=True,
                                 start=kw.pop("start", False),
                                 stop=kw.pop("stop", False), **kw)

            # Diagonal (mixed) tiles, DoubleRow-paired on their shared spans.
            # Tile j's L span is [64j, 512) (its own mixed bucket holds
            # host-pre-masked lower data), U span is [0, 64j); the mixed
            # bucket's upper part comes from the diagu strips.
            for j in range(0, TPC, 2):
                a, b = BCAP * j, BCAP * (j + 1)
                rhs2 = chunk_rhs(j, 2)
                # shared L span of the pair
                mm(acc[:, b:CAP], wv(j, 2), rhs2[:, :, b:CAP],
                   start=(j == 0), perf_mode=DR)
                # tile j's extra L strip (its own bucket)
                mm(acc[:, a:b], wv(j), chunk_rhs(j)[:, :, a:b])
                if j > 0:
                    # shared U span of the pair
                    mm(acc[:, 0:a], wv(NT + j, 2), rhs2[:, :, 0:a], perf_mode=DR)
                # tile j+1's extra U strip (tile j's bucket columns)
                mm(acc[:, a:b], wv(NT + j + 1), chunk_rhs(j + 1)[:, :, a:b])
                # upper parts of the mixed buckets themselves
                mm(acc[:, a:b], wv(NT + j), du(j))
                mm(acc[:, b : b + BCAP], wv(NT + j + 1), du(j + 1))
            for j in range(TPC, NT, 2):
                mm(acc[:, :], wv(j, 2), chunk_rhs(j, 2),
                   stop=(j == NT - 2), perf_mode=DR)

            out_sb = singles.tile([M, CAP], mybir.dt.float32)
            half = CAP // 2
            nc.vector.tensor_copy(out_sb[:, half:], acc[:, half:])
            nc.scalar.copy(out_sb[:, 0:half], acc[:, 0:half])
            nc.sync.dma_start(out=stats[:, 0:half], in_=out_sb[:, 0:half])
            nc.scalar.dma_start(out=stats[:, half:], in_=out_sb[:, half:])

    nc.compile()
    return nc


def _split_fp8(v, terms=3):
    """Split f64 vector into `terms` fp8 values summing to ~v (12 mantissa bits)."""
    out = []
    r = np.asarray(v, np.float64)
    for _ in range(terms):
        t = r.astype(FP8)
        out.append(t)
        r = r - t.astype(np.float64)
    return out


def _make_wside(outputs, targets):
    """Per-row weight table [N, 8] fp8."""
    out = np.asarray(outputs, np.float64).reshape(-1)
    pos = (np.asarray(targets).reshape(-1) != 0).astype(np.float64)
    cols = [np.ones(N, FP8), pos.astype(FP8)]
    cols += _split_fp8(pos * out)
    cols += _split_fp8(np.exp(out))
    return np.stack(cols, axis=1).astype(FP8)  # [N, 8]


def _build_wmat(wside, core):
    """Per-core weight variants [128, (64+8)*16] fp8.

    Variant j (j<64): weights for local row tile j (absolute tile (8*core+j)%64).
      j < 8  -> L-only variant (diag tiles; U-only twin stored at 64+j)
      j >= 8 -> single variant, L or U half per the tile's position vs the slab
    """
    w = np.zeros((P, NT + TPC, VW), dtype=FP8)
    for j in range(NT):
        t = (TPC * core + j) % NT
        rows = wside[t * P : (t + 1) * P, :]
        if j < TPC:
            w[:, j, 0:NW] = rows
            w[:, NT + j, NW:M] = rows
        elif j < NT - TPC * core:
            w[:, j, NW:M] = rows  # rows above slab columns -> U
        else:
            w[:, j, 0:NW] = rows  # wrapped rows below slab columns -> L
    return np.ascontiguousarray(w.reshape(P, (NT + TPC) * VW))


def _prepare(outputs, targets, node_adj, idx_node):
    """Build per-core in_maps + combine context (slot->column map, multiplicities,
    host-computed contribution of any bucket-overflow columns)."""
    node_adj = np.asarray(node_adj)
    idx = np.asarray(idx_node).reshape(-1).astype(np.int64)
    ucols, mult = np.unique(idx, return_counts=True)
    wside = _make_wside(outputs, targets)

    in_maps = []
    slot_cols = np.full((NCORES, CAP), -1, np.int64)
    overflow = []
    rows128 = np.arange(P)
    s_idx = np.arange(CAP)
    base = P * (s_idx // BCAP)  # first local row of each slot's mixed block

    for d in range(NCORES):
        lo = SLAB * d
        uc = ucols[(ucols >= lo) & (ucols < lo + SLAB)]
        cols_s = np.full(CAP, -1, np.int64)
        for b in range(TPC):
            blk = uc[(uc - lo) // P == b]
            if len(blk) > BCAP:
                overflow.extend(blk[BCAP:].tolist())
                blk = blk[:BCAP]
            cols_s[BCAP * b : BCAP * b + len(blk)] = blk
        slot_cols[d] = cols_s
        valid = cols_s >= 0

        G = (node_adj[:, np.where(valid, cols_s, 0)] != 0).astype(np.float32)
        G[:, ~valid] = 0.0
        # rotate rows: local row r = absolute row (r + 1024d) mod N
        G = np.concatenate([G[lo:], G[:lo]], axis=0)
        lc = np.where(valid, cols_s - lo, -1)  # local split row (diag) per slot
        G[lc[valid], s_idx[valid]] = 0.0       # zero the diagonal
        block = G[base[None, :] + rows128[:, None], s_idx[None, :]]  # [128, CAP]
        lrow = base[None, :] + rows128[:, None]
        diagL = np.where(lrow < lc[None, :], block, 0.0)
        diagU = np.where(lrow > lc[None, :], block, 0.0)
        G[base[None, :] + rows128[:, None], s_idx[None, :]] = diagL
        # tile-major flat layout: adjf[p, CAP*j + s] = G[128j + p, s], split
        # into one contiguous dram tensor per DMA chunk
        adjf = G.reshape(NT, P, CAP).transpose(1, 0, 2).reshape(P, NT * CAP).astype(FP8)
        im = {
            "wmat": _build_wmat(wside, d),
            "diagu": np.ascontiguousarray(diagU.astype(FP8)),
        }
        t0 = 0
        for ci, nt in enumerate(CHUNK_TILES):
            im[f"adj{ci}"] = np.ascontiguousarray(
                adjf[:, t0 * CAP : (t0 + nt) * CAP]
            )
            t0 += nt
        in_maps.append(im)

    mult_of = np.zeros(N, np.int64)
    mult_of[ucols] = mult
    over_loss = _host_cols_loss(outputs, targets, node_adj, overflow, mult_of)
    ctx = {"slot_cols": slot_cols, "mult_of": mult_of, "over_loss": over_loss}
    return in_maps, ctx


def _host_cols_loss(outputs, targets, node_adj, cols, mult_of):
    """Reference-exact loss contribution of a few columns (bucket overflow only)."""
    if not cols:
        return 0.0
    cols = np.asarray(cols, np.int64)
    out = np.asarray(outputs, np.float64).reshape(-1)
    pos = np.asarray(targets).reshape(-1) != 0
    A = node_adj[:, cols] != 0
    r = np.arange(N)[:, None]
    A = A & (r != cols[None, :])
    total = 0.0
    for mask in (A & (r < cols[None, :]), A & (r > cols[None, :])):
        cnt = mask.sum(axis=0)
        poscnt = (mask & pos[:, None]).sum(axis=0)
        sumexp = (mask * np.exp(out)[:, None]).sum(axis=0)
        poslogit = (mask * (pos * out)[:, None]).sum(axis=0)
        valid = (cnt > 0) & (poscnt == 1)
        contrib = np.where(
            valid,
            (np.log(np.maximum(sumexp, 1e-300)) - poslogit) / np.maximum(cnt, 1),
            0.0,
        )
        total += (contrib * mult_of[cols]).sum()
    return total


def _combine(stats_list, ctx):
    """Per-core stats [16, CAP] f32 -> scalar loss (f64 math)."""

    def side_contrib(x):
        cnt, poscnt = x[0], x[1]
        poslogit = x[2] + x[3] + x[4]
        sumexp = x[5] + x[6] + x[7]
        valid = (cnt > 0.5) & (np.abs(poscnt - 1.0) < 0.25)
        lse = np.log(np.where(valid, np.maximum(sumexp, 1e-300), 1.0))
        return np.where(valid, (lse - poslogit) / np.maximum(cnt, 1.0), 0.0)

    total = ctx["over_loss"]
    for d, s in enumerate(stats_list):
        x = np.asarray(s, np.float64)
        contrib = side_contrib(x[0:NW]) + side_contrib(x[NW:M])
        cols = ctx["slot_cols"][d]
        valid = cols >= 0
        total += (contrib[valid] * ctx["mult_of"][cols[valid]]).sum()
    return np.array(total, dtype=np.float32)


def _ensure_axon_hooks_stub():
    """bass_utils imports antenv.axon_hooks when tracing is requested via
    env; the module is absent on some images. Provide a no-op stub so the
    import never crashes (hook=None -> bass_utils skips tracing)."""
    import sys
    import types

    try:
        import antenv.axon_hooks  # noqa: F401
    except ImportError:
        mod = types.ModuleType("antenv.axon_hooks")
        state = {"hook": None}
        mod.set_axon_ntff_profile_hook = lambda h: state.__setitem__("hook", h)
        mod.get_axon_ntff_profile_hook = lambda: state["hook"]
        sys.modules["antenv.axon_hooks"] = mod


def _device_stats(in_maps):
    _ensure_axon_hooks_stub()
    from concourse.bass_utils import run_bass_kernel_spmd

    if "nc" not in _BASS_CACHE:
        _BASS_CACHE["nc"] = _build_bass()
    last_exc = None
    for attempt in range(4):
        try:
            res = run_bass_kernel_spmd(
                _BASS_CACHE["nc"], in_maps, core_ids=list(range(NCORES))
            )
            return [r["stats"] for r in res.results]
        except Exception as e:  # transient NRT/accelerator hiccups
            last_exc = e
            try:
                # a fresh PJRT client usually recovers a transiently
                # "unrecoverable" accelerator; mirrors a process restart
                import jax
                import jax.extend.backend as _jeb

                jax.clear_caches()
                _jeb.clear_backends()
            except Exception:
                pass
            import time

            time.sleep(2.0 * (attempt + 1))
    raise last_exc


def _sim_stats(in_maps):
    """Numpy emulation of the device kernel (same inputs), for logic validation."""
    outs = []
    for m in in_maps:
        adjf = np.concatenate(
            [m[f"adj{ci}"] for ci in range(len(CHUNK_TILES))], axis=1
        ).astype(np.float32)
        diagu = m["diagu"].astype(np.float32)
        w = m["wmat"].reshape(P, NT + TPC, VW).astype(np.float32)
        acc = np.zeros((M, CAP), np.float32)
        for j in range(NT):
            tile = adjf[:, j * CAP : (j + 1) * CAP]
            if j < TPC:
                c0 = BCAP * j
                acc[:, c0:] += w[:, j, :M].T @ tile[:, c0:]
                acc[:, :c0] += w[:, NT + j, :M].T @ tile[:, :c0]
                acc[:, c0 : c0 + BCAP] += w[:, NT + j, :M].T @ diagu[:, c0 : c0 + BCAP]
            else:
                acc += w[:, j, :M].T @ tile
        outs.append(acc)
    return outs


def kernel(outputs, targets, node_adj, idx_node, _simulate=False):
    in_maps, ctx = _prepare(outputs, targets, node_adj, idx_node)
    stats = _sim_stats(in_maps) if _simulate else _device_stats(in_maps)
    return _combine(stats, ctx)
